# revision 24
# baseline (speedup 1.0000x reference)
"""Trainium2 Bass kernel for per-variable-MLP GNN message passing.

Model (reference):
    adj  = ones(D,D) - eye(D)                       # var t cannot see itself
    h0   = leaky_relu(einsum('tij,bj->bti', w0*adjmask, x) + b0)
    h1   = leaky_relu(einsum('tij,btj->bti', w1, h0) + b1)
    out  = einsum('tij,btj->bti', w2, h1) + b2      # (B, D, O)

Sharding: variable axis t (128) split across 8 cores (16 vars each); each
core sees the full batch. Vars processed in pairs (two 64-wide MLPs stacked
to fill the 128-wide PE array); activations live transposed (feature on
partition, batch on free).

v2 structure (vs the 146us baseline):
- Batch is processed in GROUPS of 1024 (2 psum banks). The L0 psum tile is
  [128,1024] so the ScalarE Prelu epilogue runs at FD=1024, amortizing the
  per-op overhead. L1 epilogues stay FD=512 (psum budget).
- Epilogues are split three ways: ScalarE (fused Prelu+bias), VectorE
  (bias-add crossing) + GpSimd (SBUF-side leaky), per a static pattern
  tuned so all three engines finish a group at the same time.
- L2 outputs are partition-packed: each pair's stationary is an M=32
  zero-padded block so all 8 pairs accumulate into psum partitions
  [0,32); four consecutive batch chunks stack into the four 32-row
  regions of ONE psum bank. b2 is pre-filled into the bank by a K=1
  matmul (bias x ones), so a single VectorE copy evacuates 4 chunks of
  finished output, and the DMA-out is 1MB/core instead of 4MB.
- Pipeline: stage A (L0, group g), B (L1, g-1), C (L2, g-2), interleaved
  at pair granularity so the in-order PE queue never parks behind an
  epilogue.

Matmuls run in fp16 (1 col/cycle on the PE, fp32 accumulate in PSUM).
"""

import numpy as np

import concourse.bass as bass
import concourse.mybir as mybir
import concourse.tile as tile
from concourse import bacc, bass_utils

F32 = mybir.dt.float32
DT = mybir.dt.float16
NPDT = np.float16

B = 8192  # batch
D = 128  # num variables (t)
H = 64  # hidden
O = 2  # output dim per variable
NCORES = 8
TPC = D // NCORES  # vars per core = 16
NPAIR = TPC // 2  # 8
GCH = 1024  # batch group (2 psum banks for the L0 tile)
NG = B // GCH  # 8
CH = 512  # psum bank = 512 fp32
ALPHA = 0.01  # leaky_relu slope

Prelu = mybir.ActivationFunctionType.Prelu
MULT = mybir.AluOpType.mult
MAX = mybir.AluOpType.max

# Pairs with CONVERT=True store h1 as relu(z1+b1) (one fused VectorE op)
# instead of leaky; exactness is restored in L2 via
#   leaky(v) = (1-a)*relu(v) + a*v:
# their w2z stationary is pre-scaled by (1-a), an extra L2 matmul adds
# a*(W2^T W1)·h0, and a*(W2^T b1) is folded into the b2 psum pre-fill.
CONVERT = [False, False, False, False, False, False, True, True]

# Unconverted pairs' L1 (FD=1024) epilogue:
#   'sc' = ScalarE fused Prelu; 'dd' = VectorE bias-add + VectorE leaky
L1_KIND = ["sc", "sc", "sc", "dd", "dd", "dd", "cv", "cv"]

assert all(CONVERT[p] == (L1_KIND[p] == "cv") for p in range(NPAIR))


def _build_program():
    nc = bacc.Bacc(trn_type="TRN2")

    xt = nc.dram_tensor("xt", (D, B), DT, kind="ExternalInput")
    w0t = nc.dram_tensor("w0t", (D, NPAIR * 128), DT, kind="ExternalInput")
    w1bd = nc.dram_tensor("w1bd", (128, NPAIR * 128), DT, kind="ExternalInput")
    # w2z: per pair a [128,32] block; cols 4p..4p+4 hold the pair's two
    # [64,2] output blocks, the other 28 cols are zero so all pairs can
    # accumulate into psum partitions [0,32).
    w2z = nc.dram_tensor("w2z", (128, NPAIR * 32), DT, kind="ExternalInput")
    # w21z: a*(W1 @ W2z) per pair, for the relu-conversion compensation
    w21z = nc.dram_tensor("w21z", (128, NPAIR * 32), DT, kind="ExternalInput")
    b0c = nc.dram_tensor("b0c", (128, NPAIR), F32, kind="ExternalInput")
    b1c = nc.dram_tensor("b1c", (128, NPAIR), F32, kind="ExternalInput")
    # b2 as a K=1 stationary: col 32u+r = b2 for L2-row r (replicated x4)
    b2st = nc.dram_tensor("b2st", (1, 128), DT, kind="ExternalInput")
    # output rows: r = 4p + 2v + o
    ot = nc.dram_tensor("ot", (32, B), F32, kind="ExternalOutput")

    with tile.TileContext(nc) as tc:
        with (
            tc.tile_pool(name="wp", bufs=1) as wp,
            tc.tile_pool(name="hp", bufs=18) as hp,
            tc.tile_pool(name="yp", bufs=8) as yp,
            tc.tile_pool(name="obp", bufs=2) as obp,
            tc.tile_pool(name="z0p", bufs=2, space="PSUM") as z0p,
            tc.tile_pool(name="z1p", bufs=1, space="PSUM") as z1p,
            tc.tile_pool(name="z2p", bufs=2, space="PSUM") as z2p,
        ):
            xs = wp.tile([D, B], DT)
            w0s = wp.tile([D, NPAIR * 128], DT)
            w1s = wp.tile([128, NPAIR * 128], DT)
            w2s = wp.tile([128, NPAIR * 32], DT)
            w21s = wp.tile([128, NPAIR * 32], DT)
            b0s = wp.tile([128, NPAIR], F32)
            b1s = wp.tile([128, NPAIR], F32)
            b2w = wp.tile([1, 128], DT)
            ones1 = wp.tile([1, CH], DT)
            nc.sync.dma_start(xs[:, 0:GCH], xt[:, 0:GCH])
            nc.sync.dma_start(w0s[:], w0t[:])
            nc.sync.dma_start(b0s[:], b0c[:])
            nc.sync.dma_start(w1s[:], w1bd[:])
            nc.sync.dma_start(b1s[:], b1c[:])
            nc.sync.dma_start(w2s[:], w2z[:])
            nc.sync.dma_start(w21s[:], w21z[:])
            nc.sync.dma_start(b2w[:], b2st[:])
            nc.vector.memset(ones1[:], 1.0)
            xs_loaded = 1  # groups staged so far

            # PE warmup: dummy matmuls with no input-DMA dependency so the
            # HAM clock-gate reaches 8/8 while the input DMAs run.
            warm = wp.tile([128, CH], DT, name="warm")
            nc.vector.memset(warm[:], 0.0)
            wps = z1p.tile([128, GCH], F32, name="warmps", tag="z1")
            for _ in range(14):
                nc.tensor.matmul(wps[:, 0:CH], warm[:, 0:128], warm[:],
                                 start=True, stop=True)
            # preload the Prelu ACT table set during the input DMAs
            wact = wp.tile([128, 8], DT, name="wact")
            nc.scalar.activation(wact[:], wps[:, 0:8], Prelu, bias=0.0,
                                 scale=1.0, alpha=ALPHA)

            def epilogue(dst, z, bias_col, kind):
                """dst (fp16 SBUF) = leaky_relu(z + bias) ('sc'/'dd'), or
                relu(z + bias) ('cv'); z in PSUM, any width."""
                if kind == "sc":
                    nc.scalar.activation(
                        dst, z, Prelu, bias=bias_col, scale=1.0, alpha=ALPHA
                    )
                elif kind == "cv":
                    nc.vector.tensor_scalar(
                        dst, z, bias_col, 0.0, mybir.AluOpType.add, MAX
                    )
                else:
                    w = z.shape[-1]
                    y = yp.tile([128, GCH], DT, tag="y", name="y")
                    nc.vector.tensor_scalar_add(y[:, 0:w], z, bias_col)
                    nc.vector.scalar_tensor_tensor(
                        dst, y[:, 0:w], ALPHA, y[:, 0:w], MULT, MAX
                    )

            h0_tiles = [None] * NG  # 8 tiles of [128,1024] per group
            h1_tiles = [None] * NG
            z2_cur = [None]  # current 4-chunk L2 psum bank

            for k in range(NG + 2):
                if xs_loaded < NG:
                    g = xs_loaded
                    nc.sync.dma_start(
                        xs[:, g * GCH : (g + 1) * GCH], xt[:, g * GCH : (g + 1) * GCH]
                    )
                    xs_loaded += 1
                gA, gB, gC = k, k - 1, k - 2
                for p in range(NPAIR):
                    # ---- stage A: L0 pair p of group gA ----
                    if gA < NG:
                        z0 = z0p.tile([128, GCH], F32, tag="z0",
                                      name=f"z0_{gA}_{p}")
                        for hf in (0, 1):
                            nc.tensor.matmul(
                                z0[:, hf * CH : (hf + 1) * CH],
                                w0s[:, bass.ts(p, 128)],
                                xs[:, gA * GCH + hf * CH : gA * GCH + (hf + 1) * CH],
                                start=True, stop=True,
                            )
                        h0 = hp.tile([128, GCH], DT, tag="h0",
                                     name=f"h0_{gA}_{p}", bufs=26)
                        if gA == 0:
                            # pipeline-fill: split across engines (DVE is
                            # otherwise idle, halves the PE unblock latency)
                            nc.scalar.activation(
                                h0[:, 0:CH], z0[:, 0:CH], Prelu,
                                bias=b0s[:, p : p + 1], scale=1.0, alpha=ALPHA,
                            )
                            epilogue(h0[:, CH:GCH], z0[:, CH:GCH],
                                     b0s[:, p : p + 1], "dd")
                        else:
                            nc.scalar.activation(
                                h0[:], z0[:], Prelu, bias=b0s[:, p : p + 1],
                                scale=1.0, alpha=ALPHA,
                            )
                        if p == 0:
                            h0_tiles[gA] = [None] * NPAIR
                        h0_tiles[gA][p] = h0

                    # ---- stage B: L1 pair p of group gB ----
                    if 0 <= gB < NG:
                        h1 = hp.tile([128, GCH], DT, tag="h1",
                                     name=f"h1_{gB}_{p}", bufs=18)
                        z1 = z1p.tile([128, GCH], F32, tag="z1",
                                      name=f"z1_{gB}_{p}")
                        for hf in (0, 1):
                            nc.tensor.matmul(
                                z1[:, hf * CH : (hf + 1) * CH],
                                w1s[:, bass.ts(p, 128)],
                                h0_tiles[gB][p][:, hf * CH : (hf + 1) * CH],
                                start=True, stop=True,
                            )
                        epilogue(h1[:], z1[:], b1s[:, p : p + 1], L1_KIND[p])
                        if p == 0:
                            h1_tiles[gB] = [None] * NPAIR
                        h1_tiles[gB][p] = h1

                    # ---- stage C: L2 pair p of group gC ----
                    if 0 <= gC < NG:
                        for hf in (0, 1):
                            chunk = 2 * gC + hf
                            u = chunk % 4
                            if u == 0 and p == 0:
                                z2 = z2p.tile([128, CH], F32, tag="z2",
                                              name=f"z2_{chunk // 4}")
                                z2_cur[0] = z2
                                # bias pre-fill: z2[m,:] = b2[m], sets
                                # has_written for the whole bank
                                nc.tensor.matmul(
                                    z2[:], b2w[0:1, :], ones1[0:1, :],
                                    start=True, stop=False,
                                )
                            last = u == 3 and p == NPAIR - 1
                            nc.tensor.matmul(
                                z2_cur[0][32 * u : 32 * u + 32, :],
                                w2s[:, bass.ts(p, 32)],
                                h1_tiles[gC][p][:, hf * CH : (hf + 1) * CH],
                                start=False,
                                stop=last and not CONVERT[p],
                                tile_position=(0, 32 * u),
                            )
                            if CONVERT[p]:
                                # + a*(W2^T W1)·h0 (relu-conversion term)
                                nc.tensor.matmul(
                                    z2_cur[0][32 * u : 32 * u + 32, :],
                                    w21s[:, bass.ts(p, 32)],
                                    h0_tiles[gC][p][:, hf * CH : (hf + 1) * CH],
                                    start=False,
                                    stop=last,
                                    tile_position=(0, 32 * u),
                                )
                        if p == NPAIR - 1:
                            h1_tiles[gC] = None
                            h0_tiles[gC] = None

                # ---- stage C evac: one copy + 4 DMAs per 4 chunks ----
                if 0 <= gC < NG and gC % 2 == 1:
                    bank = gC // 2  # chunks 4*bank .. 4*bank+3
                    ob = obp.tile([128, CH], F32, tag="ob", name=f"ob_{bank}")
                    nc.vector.tensor_copy(ob[:], z2_cur[0][:])
                    for u in range(4):
                        c = 4 * bank + u
                        nc.sync.dma_start(
                            ot[0:32, c * CH : (c + 1) * CH],
                            ob[32 * u : 32 * u + 32, :],
                        )

    nc.finalize()
    return nc


_prog = None


def _get_program():
    global _prog
    if _prog is None:
        _prog = _build_program()
    return _prog


def _shard_inputs(x, w0, w1, w2, b0, b1, b2):
    """Host-side relayout + t-sharding. Returns list of 8 in_maps."""
    x = np.asarray(x, np.float32)
    w0 = np.array(w0, np.float32)  # copy: we zero the adjacency diagonal
    w1 = np.asarray(w1, np.float32)
    w2 = np.asarray(w2, np.float32)
    b0 = np.asarray(b0, np.float32)
    b1 = np.asarray(b1, np.float32)
    b2 = np.asarray(b2, np.float32)

    # adjacency mask: variable t cannot see itself -> w0[t, :, t] = 0
    ar = np.arange(D)
    w0[ar, :, ar] = 0.0

    xt = np.ascontiguousarray(x.T).astype(NPDT)  # (128, 8192)

    in_maps = []
    for c in range(NCORES):
        ts_ = slice(c * TPC, (c + 1) * TPC)
        w0c, w1c, w2c = w0[ts_], w1[ts_], w2[ts_]
        b0cc, b1cc, b2cc = b0[ts_], b1[ts_], b2[ts_]

        # w0t: (128 j, pair*128 + [ta's 64 i | tb's 64 i])
        w0T = w0c.transpose(0, 2, 1)  # (16, 128 j, 64 i)
        w0t_ = np.ascontiguousarray(
            w0T.reshape(NPAIR, 2, D, H).transpose(2, 0, 1, 3).reshape(D, NPAIR * 128)
        ).astype(NPDT)

        # w1bd: per-pair 128x128 block-diagonal blocks
        bd1 = np.zeros((NPAIR, 128, 128), np.float32)
        for p in range(NPAIR):
            bd1[p, 0:H, 0:H] = w1c[2 * p].T
            bd1[p, H:128, H:128] = w1c[2 * p + 1].T
        w1bd_ = np.ascontiguousarray(
            bd1.transpose(1, 0, 2).reshape(128, NPAIR * 128)
        ).astype(NPDT)

        b0c_ = np.ascontiguousarray(b0cc.reshape(NPAIR, 128).T).astype(np.float32)
        b1c_ = np.ascontiguousarray(b1cc.reshape(NPAIR, 128).T).astype(np.float32)

        # w2z: per pair [128, 32], nonzero only in cols 4p..4p+4.
        # Converted pairs are pre-scaled by (1-a): leaky = (1-a)relu + a*id.
        z2w = np.zeros((NPAIR, 128, 32), np.float32)
        for p in range(NPAIR):
            z2w[p, 0:H, 4 * p : 4 * p + 2] = w2c[2 * p].T  # (64, 2)
            z2w[p, H:128, 4 * p + 2 : 4 * p + 4] = w2c[2 * p + 1].T
        # w21z: a * (W1bd @ w2z) per pair (compensation stationary), and the
        # b2 correction a * (w2z^T b1) per pair, both on UNSCALED w2z.
        z2w1 = np.einsum("pjk,pkm->pjm", bd1, z2w) * ALPHA  # (NPAIR,128,32)
        b2corr = np.einsum("pkm,pk->m", z2w,
                           np.where(CONVERT, ALPHA, 0.0)[:, None]
                           * b1cc.reshape(NPAIR, 128))  # (32,)
        for p in range(NPAIR):
            if CONVERT[p]:
                z2w[p] *= 1.0 - ALPHA
        w2z_ = np.ascontiguousarray(
            z2w.transpose(1, 0, 2).reshape(128, NPAIR * 32)
        ).astype(NPDT)
        w21z_ = np.ascontiguousarray(
            z2w1.transpose(1, 0, 2).reshape(128, NPAIR * 32)
        ).astype(NPDT)

        # b2st: K=1 stationary, col 32u + (4p+2v+o) = b2[2p+v, o], u=0..3
        b2row = b2cc.reshape(32) + b2corr  # row r = 4p+2v+o
        b2st_ = np.tile(b2row, 4)[None, :].astype(NPDT)  # (1, 128)

        in_maps.append(
            {
                "xt": xt,
                "w0t": w0t_,
                "w1bd": w1bd_,
                "w2z": w2z_,
                "w21z": w21z_,
                "b0c": b0c_,
                "b1c": b1c_,
                "b2st": b2st_,
            }
        )
    return in_maps


def _unshard_outputs(results):
    out = np.empty((B, D, O), np.float32)
    for c in range(NCORES):
        ot = results[c]["ot"]  # (32, 8192): row = 4p + 2v + o
        blk = ot.reshape(NPAIR, 2, O, B).transpose(3, 0, 1, 2).reshape(B, TPC, O)
        out[:, c * TPC : (c + 1) * TPC, :] = blk
    return out


def kernel(x, w0, w1, w2, b0, b1, b2):
    nc = _get_program()
    in_maps = _shard_inputs(x, w0, w1, w2, b0, b1, b2)
    res = bass_utils.run_bass_kernel_spmd(nc, in_maps, core_ids=list(range(NCORES)))
    return _unshard_outputs(res.results)


# revision 28
# speedup vs baseline: 1.3270x; 1.3270x over previous
"""Trainium2 Bass kernel for per-variable-MLP GNN message passing.

Model (reference):
    adj  = ones(D,D) - eye(D)                       # var t cannot see itself
    h0   = leaky_relu(einsum('tij,bj->bti', w0*adjmask, x) + b0)
    h1   = leaky_relu(einsum('tij,btj->bti', w1, h0) + b1)
    out  = einsum('tij,btj->bti', w2, h1) + b2      # (B, D, O)

Sharding: variable axis t (128) split across 8 cores (16 vars each); each
core sees the full batch. Vars processed in pairs (two 64-wide MLPs stacked
to fill the 128-wide PE array); activations live transposed (feature on
partition, batch on free).

v2 structure (vs the 146us baseline):
- Batch is processed in GROUPS of 1024 (2 psum banks). The L0 psum tile is
  [128,1024] so the ScalarE Prelu epilogue runs at FD=1024, amortizing the
  per-op overhead. L1 epilogues stay FD=512 (psum budget).
- Epilogues are split three ways: ScalarE (fused Prelu+bias), VectorE
  (bias-add crossing) + GpSimd (SBUF-side leaky), per a static pattern
  tuned so all three engines finish a group at the same time.
- L2 outputs are partition-packed: each pair's stationary is an M=32
  zero-padded block so all 8 pairs accumulate into psum partitions
  [0,32); four consecutive batch chunks stack into the four 32-row
  regions of ONE psum bank. b2 is pre-filled into the bank by a K=1
  matmul (bias x ones), so a single VectorE copy evacuates 4 chunks of
  finished output, and the DMA-out is 1MB/core instead of 4MB.
- Pipeline: stage A (L0, group g), B (L1, g-1), C (L2, g-2), interleaved
  at pair granularity so the in-order PE queue never parks behind an
  epilogue.

Matmuls run in fp16 (1 col/cycle on the PE, fp32 accumulate in PSUM).
"""

import numpy as np

import concourse.bass as bass
import concourse.mybir as mybir
import concourse.tile as tile
from concourse import bacc, bass_utils

F32 = mybir.dt.float32
DT = mybir.dt.float16
NPDT = np.float16

B = 8192  # batch
D = 128  # num variables (t)
H = 64  # hidden
O = 2  # output dim per variable
NCORES = 8
TPC = D // NCORES  # vars per core = 16
NPAIR = TPC // 2  # 8
GCH = 1024  # batch group (2 psum banks for the L0 tile)
NG = B // GCH  # 8
CH = 512  # psum bank = 512 fp32
ALPHA = 0.01  # leaky_relu slope

Prelu = mybir.ActivationFunctionType.Prelu
MULT = mybir.AluOpType.mult
MAX = mybir.AluOpType.max

# Pairs with CONVERT=True store h1 as relu(z1+b1) (one fused VectorE op)
# instead of leaky; exactness is restored in L2 via
#   leaky(v) = (1-a)*relu(v) + a*v:
# their w2z stationary is pre-scaled by (1-a), an extra L2 matmul adds
# a*(W2^T W1)·h0, and a*(W2^T b1) is folded into the b2 psum pre-fill.
CONVERT = [False, False, False, False, True, True, True, True]

# Unconverted pairs' L1 (FD=512) epilogue per half:
#   'sc' = ScalarE fused Prelu; 'dd' = VectorE bias-add + VectorE leaky
SMALL_KIND = {0: "sc", 1: "dd"}


def _build_program():
    nc = bacc.Bacc(trn_type="TRN2")

    xt = nc.dram_tensor("xt", (D, B), DT, kind="ExternalInput")
    w0t = nc.dram_tensor("w0t", (D, NPAIR * 128), DT, kind="ExternalInput")
    w1bd = nc.dram_tensor("w1bd", (128, NPAIR * 128), DT, kind="ExternalInput")
    # w2z: per pair a [128,32] block; cols 4p..4p+4 hold the pair's two
    # [64,2] output blocks, the other 28 cols are zero so all pairs can
    # accumulate into psum partitions [0,32).
    w2z = nc.dram_tensor("w2z", (128, NPAIR * 32), DT, kind="ExternalInput")
    # w21z: a*(W1 @ W2z) per pair, for the relu-conversion compensation
    w21z = nc.dram_tensor("w21z", (128, NPAIR * 32), DT, kind="ExternalInput")
    b0c = nc.dram_tensor("b0c", (128, NPAIR), F32, kind="ExternalInput")
    b1c = nc.dram_tensor("b1c", (128, NPAIR), F32, kind="ExternalInput")
    # b2 as a K=1 stationary: col 32u+r = b2 for L2-row r (replicated x4)
    b2st = nc.dram_tensor("b2st", (1, 128), DT, kind="ExternalInput")
    # output rows: r = 4p + 2v + o
    ot = nc.dram_tensor("ot", (32, B), F32, kind="ExternalOutput")

    with tile.TileContext(nc) as tc:
        with (
            tc.tile_pool(name="wp", bufs=1) as wp,
            tc.tile_pool(name="hp", bufs=18) as hp,
            tc.tile_pool(name="yp", bufs=8) as yp,
            tc.tile_pool(name="obp", bufs=2) as obp,
            tc.tile_pool(name="z0p", bufs=2, space="PSUM") as z0p,
            tc.tile_pool(name="z1p", bufs=3, space="PSUM") as z1p,
            tc.tile_pool(name="z2p", bufs=1, space="PSUM") as z2p,
        ):
            xs = wp.tile([D, B], DT)
            w0s = wp.tile([D, NPAIR * 128], DT)
            w1s = wp.tile([128, NPAIR * 128], DT)
            w2s = wp.tile([128, NPAIR * 32], DT)
            w21s = wp.tile([128, NPAIR * 32], DT)
            b0s = wp.tile([128, NPAIR], F32)
            b1s = wp.tile([128, NPAIR], F32)
            b2w = wp.tile([1, 128], DT)
            ones1 = wp.tile([1, CH], DT)
            nc.sync.dma_start(xs[:, 0:GCH], xt[:, 0:GCH])
            nc.sync.dma_start(w0s[:], w0t[:])
            nc.sync.dma_start(b0s[:], b0c[:])
            nc.sync.dma_start(w1s[:], w1bd[:])
            nc.sync.dma_start(b1s[:], b1c[:])
            nc.sync.dma_start(w2s[:], w2z[:])
            nc.sync.dma_start(w21s[:], w21z[:])
            nc.sync.dma_start(b2w[:], b2st[:])
            nc.vector.memset(ones1[:], 1.0)
            xs_loaded = 1  # groups staged so far

            # PE warmup: dummy matmuls with no input-DMA dependency so the
            # HAM clock-gate reaches 8/8 while the input DMAs run.
            warm = wp.tile([128, CH], DT, name="warm")
            nc.vector.memset(warm[:], 0.0)
            wps = z1p.tile([128, CH], F32, name="warmps", tag="z1")
            for _ in range(14):
                nc.tensor.matmul(wps[:], warm[:, 0:128], warm[:],
                                 start=True, stop=True)
            # preload the Prelu ACT table set during the input DMAs
            wact = wp.tile([128, 8], DT, name="wact")
            nc.scalar.activation(wact[:], wps[:, 0:8], Prelu, bias=0.0,
                                 scale=1.0, alpha=ALPHA)

            def epilogue(dst, z, bias_col, kind):
                """dst (fp16 SBUF) = leaky_relu(z + bias) ('sc'/'dd'), or
                relu(z + bias) ('cv'); z in PSUM, any width."""
                if kind == "sc":
                    nc.scalar.activation(
                        dst, z, Prelu, bias=bias_col, scale=1.0, alpha=ALPHA
                    )
                elif kind == "cv":
                    nc.vector.tensor_scalar(
                        dst, z, bias_col, 0.0, mybir.AluOpType.add, MAX
                    )
                else:
                    w = z.shape[-1]
                    y = yp.tile([128, GCH], DT, tag="y", name="y")
                    nc.vector.tensor_scalar_add(y[:, 0:w], z, bias_col)
                    nc.vector.scalar_tensor_tensor(
                        dst, y[:, 0:w], ALPHA, y[:, 0:w], MULT, MAX
                    )

            h0_tiles = [None] * NG  # 8 tiles of [128,1024] per group
            h1_tiles = [None] * NG
            z2_cur = [None]  # current 4-chunk L2 psum bank

            for k in range(NG + 2):
                if xs_loaded < NG:
                    g = xs_loaded
                    nc.sync.dma_start(
                        xs[:, g * GCH : (g + 1) * GCH], xt[:, g * GCH : (g + 1) * GCH]
                    )
                    xs_loaded += 1
                gA, gB, gC = k, k - 1, k - 2
                for p in range(NPAIR):
                    # ---- stage A: L0 pair p of group gA ----
                    if gA < NG:
                        z0 = z0p.tile([128, GCH], F32, tag="z0",
                                      name=f"z0_{gA}_{p}")
                        for hf in (0, 1):
                            nc.tensor.matmul(
                                z0[:, hf * CH : (hf + 1) * CH],
                                w0s[:, bass.ts(p, 128)],
                                xs[:, gA * GCH + hf * CH : gA * GCH + (hf + 1) * CH],
                                start=True, stop=True,
                            )
                        h0 = hp.tile([128, GCH], DT, tag="h0",
                                     name=f"h0_{gA}_{p}", bufs=26)
                        if gA == 0:
                            # pipeline-fill: split across engines (DVE is
                            # otherwise idle, halves the PE unblock latency)
                            nc.scalar.activation(
                                h0[:, 0:CH], z0[:, 0:CH], Prelu,
                                bias=b0s[:, p : p + 1], scale=1.0, alpha=ALPHA,
                            )
                            epilogue(h0[:, CH:GCH], z0[:, CH:GCH],
                                     b0s[:, p : p + 1], "dd")
                        else:
                            nc.scalar.activation(
                                h0[:], z0[:], Prelu, bias=b0s[:, p : p + 1],
                                scale=1.0, alpha=ALPHA,
                            )
                        if p == 0:
                            h0_tiles[gA] = [None] * NPAIR
                        h0_tiles[gA][p] = h0

                    # ---- stage B: L1 pair p of group gB ----
                    if 0 <= gB < NG:
                        h1 = hp.tile([128, GCH], DT, tag="h1",
                                     name=f"h1_{gB}_{p}", bufs=18)
                        for hf in (0, 1):
                            z1 = z1p.tile([128, CH], F32, tag="z1",
                                          name=f"z1_{gB}_{p}_{hf}")
                            nc.tensor.matmul(
                                z1[:], w1s[:, bass.ts(p, 128)],
                                h0_tiles[gB][p][:, hf * CH : (hf + 1) * CH],
                                start=True, stop=True,
                            )
                            epilogue(
                                h1[:, hf * CH : (hf + 1) * CH], z1[:],
                                b1s[:, p : p + 1],
                                "cv" if CONVERT[p] else SMALL_KIND[hf],
                            )
                        if p == 0:
                            h1_tiles[gB] = [None] * NPAIR
                        h1_tiles[gB][p] = h1

                    # ---- stage C: L2 pair p of group gC ----
                    if 0 <= gC < NG:
                        for hf in (0, 1):
                            chunk = 2 * gC + hf
                            u = chunk % 4
                            if u == 0 and p == 0:
                                z2 = z2p.tile([128, CH], F32, tag="z2",
                                              name=f"z2_{chunk // 4}")
                                z2_cur[0] = z2
                                # bias pre-fill: z2[m,:] = b2[m], sets
                                # has_written for the whole bank
                                nc.tensor.matmul(
                                    z2[:], b2w[0:1, :], ones1[0:1, :],
                                    start=True, stop=False,
                                )
                            last = u == 3 and p == NPAIR - 1
                            nc.tensor.matmul(
                                z2_cur[0][32 * u : 32 * u + 32, :],
                                w2s[:, bass.ts(p, 32)],
                                h1_tiles[gC][p][:, hf * CH : (hf + 1) * CH],
                                start=False,
                                stop=last and not CONVERT[p],
                                tile_position=(0, 32 * u),
                            )
                            if CONVERT[p]:
                                # + a*(W2^T W1)·h0 (relu-conversion term)
                                nc.tensor.matmul(
                                    z2_cur[0][32 * u : 32 * u + 32, :],
                                    w21s[:, bass.ts(p, 32)],
                                    h0_tiles[gC][p][:, hf * CH : (hf + 1) * CH],
                                    start=False,
                                    stop=last,
                                    tile_position=(0, 32 * u),
                                )
                        if p == NPAIR - 1:
                            h1_tiles[gC] = None
                            h0_tiles[gC] = None

                # ---- stage C evac: one copy + 4 DMAs per 4 chunks ----
                if 0 <= gC < NG and gC % 2 == 1:
                    bank = gC // 2  # chunks 4*bank .. 4*bank+3
                    ob = obp.tile([128, CH], F32, tag="ob", name=f"ob_{bank}")
                    nc.vector.tensor_copy(ob[:], z2_cur[0][:])
                    for u in range(4):
                        c = 4 * bank + u
                        nc.sync.dma_start(
                            ot[0:32, c * CH : (c + 1) * CH],
                            ob[32 * u : 32 * u + 32, :],
                        )

    nc.finalize()
    return nc


_prog = None


def _get_program():
    global _prog
    if _prog is None:
        _prog = _build_program()
    return _prog


def _shard_inputs(x, w0, w1, w2, b0, b1, b2):
    """Host-side relayout + t-sharding. Returns list of 8 in_maps."""
    x = np.asarray(x, np.float32)
    w0 = np.array(w0, np.float32)  # copy: we zero the adjacency diagonal
    w1 = np.asarray(w1, np.float32)
    w2 = np.asarray(w2, np.float32)
    b0 = np.asarray(b0, np.float32)
    b1 = np.asarray(b1, np.float32)
    b2 = np.asarray(b2, np.float32)

    # adjacency mask: variable t cannot see itself -> w0[t, :, t] = 0
    ar = np.arange(D)
    w0[ar, :, ar] = 0.0

    xt = np.ascontiguousarray(x.T).astype(NPDT)  # (128, 8192)

    in_maps = []
    for c in range(NCORES):
        ts_ = slice(c * TPC, (c + 1) * TPC)
        w0c, w1c, w2c = w0[ts_], w1[ts_], w2[ts_]
        b0cc, b1cc, b2cc = b0[ts_], b1[ts_], b2[ts_]

        # w0t: (128 j, pair*128 + [ta's 64 i | tb's 64 i])
        w0T = w0c.transpose(0, 2, 1)  # (16, 128 j, 64 i)
        w0t_ = np.ascontiguousarray(
            w0T.reshape(NPAIR, 2, D, H).transpose(2, 0, 1, 3).reshape(D, NPAIR * 128)
        ).astype(NPDT)

        # w1bd: per-pair 128x128 block-diagonal blocks
        bd1 = np.zeros((NPAIR, 128, 128), np.float32)
        for p in range(NPAIR):
            bd1[p, 0:H, 0:H] = w1c[2 * p].T
            bd1[p, H:128, H:128] = w1c[2 * p + 1].T
        w1bd_ = np.ascontiguousarray(
            bd1.transpose(1, 0, 2).reshape(128, NPAIR * 128)
        ).astype(NPDT)

        b0c_ = np.ascontiguousarray(b0cc.reshape(NPAIR, 128).T).astype(np.float32)
        b1c_ = np.ascontiguousarray(b1cc.reshape(NPAIR, 128).T).astype(np.float32)

        # w2z: per pair [128, 32], nonzero only in cols 4p..4p+4.
        # Converted pairs are pre-scaled by (1-a): leaky = (1-a)relu + a*id.
        z2w = np.zeros((NPAIR, 128, 32), np.float32)
        for p in range(NPAIR):
            z2w[p, 0:H, 4 * p : 4 * p + 2] = w2c[2 * p].T  # (64, 2)
            z2w[p, H:128, 4 * p + 2 : 4 * p + 4] = w2c[2 * p + 1].T
        # w21z: a * (W1bd @ w2z) per pair (compensation stationary), and the
        # b2 correction a * (w2z^T b1) per pair, both on UNSCALED w2z.
        z2w1 = np.einsum("pjk,pkm->pjm", bd1, z2w) * ALPHA  # (NPAIR,128,32)
        b2corr = np.einsum("pkm,pk->m", z2w,
                           np.where(CONVERT, ALPHA, 0.0)[:, None]
                           * b1cc.reshape(NPAIR, 128))  # (32,)
        for p in range(NPAIR):
            if CONVERT[p]:
                z2w[p] *= 1.0 - ALPHA
        w2z_ = np.ascontiguousarray(
            z2w.transpose(1, 0, 2).reshape(128, NPAIR * 32)
        ).astype(NPDT)
        w21z_ = np.ascontiguousarray(
            z2w1.transpose(1, 0, 2).reshape(128, NPAIR * 32)
        ).astype(NPDT)

        # b2st: K=1 stationary, col 32u + (4p+2v+o) = b2[2p+v, o], u=0..3
        b2row = b2cc.reshape(32) + b2corr  # row r = 4p+2v+o
        b2st_ = np.tile(b2row, 4)[None, :].astype(NPDT)  # (1, 128)

        in_maps.append(
            {
                "xt": xt,
                "w0t": w0t_,
                "w1bd": w1bd_,
                "w2z": w2z_,
                "w21z": w21z_,
                "b0c": b0c_,
                "b1c": b1c_,
                "b2st": b2st_,
            }
        )
    return in_maps


def _unshard_outputs(results):
    out = np.empty((B, D, O), np.float32)
    for c in range(NCORES):
        ot = results[c]["ot"]  # (32, 8192): row = 4p + 2v + o
        blk = ot.reshape(NPAIR, 2, O, B).transpose(3, 0, 1, 2).reshape(B, TPC, O)
        out[:, c * TPC : (c + 1) * TPC, :] = blk
    return out


def kernel(x, w0, w1, w2, b0, b1, b2):
    nc = _get_program()
    in_maps = _shard_inputs(x, w0, w1, w2, b0, b1, b2)
    res = bass_utils.run_bass_kernel_spmd(nc, in_maps, core_ids=list(range(NCORES)))
    return _unshard_outputs(res.results)


# revision 29
# speedup vs baseline: 1.4020x; 1.0565x over previous
"""Trainium2 Bass kernel for per-variable-MLP GNN message passing.

Model (reference):
    adj  = ones(D,D) - eye(D)                       # var t cannot see itself
    h0   = leaky_relu(einsum('tij,bj->bti', w0*adjmask, x) + b0)
    h1   = leaky_relu(einsum('tij,btj->bti', w1, h0) + b1)
    out  = einsum('tij,btj->bti', w2, h1) + b2      # (B, D, O)

Sharding: variable axis t (128) split across 8 cores (16 vars each); each
core sees the full batch. Vars processed in pairs (two 64-wide MLPs stacked
to fill the 128-wide PE array); activations live transposed (feature on
partition, batch on free).

v5 structure (vs the 146us baseline):
- Batch is processed in GROUPS of 1024 (2 psum banks); the L0 psum tile is
  [128,1024] so the ScalarE Prelu epilogue runs at FD=1024 (amortized
  per-op overhead). L1 psums stay [128,512].
- ALL pairs use the relu decomposition  leaky(v) = (1-a)relu(v) + a*v:
  h1 is stored as relu(z1+b1) -- a single fused op on EITHER VectorE
  (tensor_scalar add+max) or ScalarE (Relu activation), splitting the
  PSUM->SBUF crossing load across both engines; the a*W2^T(z1+b1) term
  is restored inside L2 via an extra matmul on h0 with the host-folded
  stationary a*(W1@W2z), plus a bias correction in the b2 pre-fill.
- L2 exploits PE column-tiling concurrency (4 col-groups issue within
  ~4ns of each other, measured 3x): pair p lands in col group p%4, and
  four consecutive batch chunks stack in rows 8s of each group, so one
  psum bank holds 4 chunks of finished output, fully using all 128
  partitions. Data + compensation matmuls run as 4-MM quads; one copy +
  one DMA per bank evacuates 4 chunks (1MB/core total output traffic).
- b2 (with the relu-conversion correction) is pre-filled into the L2
  bank by a K=1 matmul so the evacuation is a plain copy.
- Pipeline: stage A (L0, group g), B (L1, g-1) interleaved at pair
  granularity, then the stage C (L2, g-2) quad block, so the in-order PE
  queue never parks behind an epilogue.

Matmuls run in fp16 (1 col/cycle on the PE, fp32 accumulate in PSUM).
"""

import numpy as np

import concourse.bass as bass
import concourse.mybir as mybir
import concourse.tile as tile
from concourse import bacc, bass_utils

F32 = mybir.dt.float32
DT = mybir.dt.float16
NPDT = np.float16

B = 8192  # batch
D = 128  # num variables (t)
H = 64  # hidden
O = 2  # output dim per variable
NCORES = 8
TPC = D // NCORES  # vars per core = 16
NPAIR = TPC // 2  # 8
GCH = 1024  # batch group (2 psum banks for the L0 tile)
NG = B // GCH  # 8
CH = 512  # psum bank = 512 fp32
NBANK = B // (4 * CH)  # L2 output banks (4 chunks each) = 4
ALPHA = 0.01  # leaky_relu slope

Prelu = mybir.ActivationFunctionType.Prelu
Relu = mybir.ActivationFunctionType.Relu
ADD = mybir.AluOpType.add
MULT = mybir.AluOpType.mult
MAX = mybir.AluOpType.max

# h1 relu-crossing halves routed to ScalarE (the rest go to VectorE)
CV_SC = {(0, 0), (4, 1)}
# group-0 L0 epilogues routed to VectorE (pipeline fill; DVE idle then)
G0_DD = {5, 6}


def _build_program():
    nc = bacc.Bacc(trn_type="TRN2")

    xt = nc.dram_tensor("xt", (D, B), DT, kind="ExternalInput")
    w0t = nc.dram_tensor("w0t", (D, NPAIR * 128), DT, kind="ExternalInput")
    w1bd = nc.dram_tensor("w1bd", (128, NPAIR * 128), DT, kind="ExternalInput")
    # w2z / w21z: per (chunk-slot s, pair p) a [128,32] block, nonzero only
    # in cols 8s+4(p//4)+0..4, so pair p's output lands at psum partition
    # 32*(p%4) + 8s + 4*(p//4) + r when issued at tile_position (0,32*(p%4)).
    w2z = nc.dram_tensor("w2z", (128, 4 * NPAIR * 32), DT, kind="ExternalInput")
    w21z = nc.dram_tensor("w21z", (128, 4 * NPAIR * 32), DT, kind="ExternalInput")
    b0c = nc.dram_tensor("b0c", (128, NPAIR), F32, kind="ExternalInput")
    b1c = nc.dram_tensor("b1c", (128, NPAIR), F32, kind="ExternalInput")
    # b2 (+ relu-conversion correction) as a K=1 matmul stationary
    b2st = nc.dram_tensor("b2st", (1, 128), DT, kind="ExternalInput")
    # output: bank b (4 chunks) -> cols [512b, 512b+512); host unscrambles
    ot = nc.dram_tensor("ot", (128, NBANK * CH), F32, kind="ExternalOutput")

    with tile.TileContext(nc) as tc:
        with (
            tc.tile_pool(name="wp", bufs=1) as wp,
            tc.tile_pool(name="hp", bufs=18) as hp,
            tc.tile_pool(name="yp", bufs=4) as yp,
            tc.tile_pool(name="obp", bufs=2) as obp,
            tc.tile_pool(name="z0p", bufs=2, space="PSUM") as z0p,
            tc.tile_pool(name="z1p", bufs=3, space="PSUM") as z1p,
            tc.tile_pool(name="z2p", bufs=1, space="PSUM") as z2p,
        ):
            xs = wp.tile([D, B], DT)
            w0s = wp.tile([D, NPAIR * 128], DT)
            w1s = wp.tile([128, NPAIR * 128], DT)
            w2s = wp.tile([128, 4 * NPAIR * 32], DT)
            w21s = wp.tile([128, 4 * NPAIR * 32], DT)
            b0s = wp.tile([128, NPAIR], F32)
            b1s = wp.tile([128, NPAIR], F32)
            b2w = wp.tile([1, 128], DT)
            ones1 = wp.tile([1, CH], DT)
            nc.sync.dma_start(xs[:, 0:GCH], xt[:, 0:GCH])
            nc.sync.dma_start(w0s[:], w0t[:])
            nc.sync.dma_start(b0s[:], b0c[:])
            nc.sync.dma_start(w1s[:], w1bd[:])
            nc.sync.dma_start(b1s[:], b1c[:])
            nc.sync.dma_start(w2s[:], w2z[:])
            nc.sync.dma_start(w21s[:], w21z[:])
            nc.sync.dma_start(b2w[:], b2st[:])
            nc.vector.memset(ones1[:], 1.0)
            xs_loaded = 1  # groups staged so far

            # PE warmup: dummy matmuls with no input-DMA dependency so the
            # HAM clock-gate reaches 8/8 while the input DMAs run.
            warm = wp.tile([128, CH], DT, name="warm")
            nc.vector.memset(warm[:], 0.0)
            wps = z1p.tile([128, CH], F32, name="warmps", tag="z1")
            for _ in range(8):
                nc.tensor.matmul(wps[:], warm[:, 0:128], warm[:], start=True,
                                 stop=True)
            # preload the Prelu ACT table set during the input DMAs
            wact = wp.tile([128, 8], DT, name="wact")
            nc.scalar.activation(wact[:], wps[:, 0:8], Prelu, bias=0.0,
                                 scale=1.0, alpha=ALPHA)

            def leaky_big(dst, z, bias_col):
                """dst = leaky_relu(z + bias) on VectorE (2 ops)."""
                w = z.shape[-1]
                y = yp.tile([128, GCH], DT, tag="y", name="y")
                nc.vector.tensor_scalar_add(y[:, 0:w], z, bias_col)
                nc.vector.scalar_tensor_tensor(dst, y[:, 0:w], ALPHA, y[:, 0:w],
                                               MULT, MAX)

            h0_tiles = [None] * NG  # 8 tiles of [128,1024] per group
            h1_tiles = [None] * NG
            z2_cur = [None]  # current 4-chunk L2 psum bank

            for k in range(NG + 2):
                if xs_loaded < NG:
                    g = xs_loaded
                    nc.sync.dma_start(
                        xs[:, g * GCH : (g + 1) * GCH], xt[:, g * GCH : (g + 1) * GCH]
                    )
                    xs_loaded += 1
                gA, gB, gC = k, k - 1, k - 2
                for p in range(NPAIR):
                    # ---- stage A: L0 pair p of group gA ----
                    if gA < NG:
                        z0 = z0p.tile([128, GCH], F32, tag="z0",
                                      name=f"z0_{gA}_{p}")
                        for hf in (0, 1):
                            nc.tensor.matmul(
                                z0[:, hf * CH : (hf + 1) * CH],
                                w0s[:, bass.ts(p, 128)],
                                xs[:, gA * GCH + hf * CH : gA * GCH + (hf + 1) * CH],
                                start=True, stop=True,
                            )
                        h0 = hp.tile([128, GCH], DT, tag="h0",
                                     name=f"h0_{gA}_{p}", bufs=26)
                        if gA == 0 and p in G0_DD:
                            leaky_big(h0[:], z0[:], b0s[:, p : p + 1])
                        else:
                            nc.scalar.activation(
                                h0[:], z0[:], Prelu, bias=b0s[:, p : p + 1],
                                scale=1.0, alpha=ALPHA,
                            )
                        if p == 0:
                            h0_tiles[gA] = [None] * NPAIR
                        h0_tiles[gA][p] = h0

                    # ---- stage B: L1 pair p of group gB (relu crossing) ----
                    if 0 <= gB < NG:
                        h1 = hp.tile([128, GCH], DT, tag="h1",
                                     name=f"h1_{gB}_{p}", bufs=18)
                        for hf in (0, 1):
                            z1 = z1p.tile([128, CH], F32, tag="z1",
                                          name=f"z1_{gB}_{p}_{hf}")
                            nc.tensor.matmul(
                                z1[:], w1s[:, bass.ts(p, 128)],
                                h0_tiles[gB][p][:, hf * CH : (hf + 1) * CH],
                                start=True, stop=True,
                            )
                            dst = h1[:, hf * CH : (hf + 1) * CH]
                            if (p, hf) in CV_SC:
                                nc.scalar.activation(
                                    dst, z1[:], Relu, bias=b1s[:, p : p + 1],
                                    scale=1.0,
                                )
                            else:
                                nc.vector.tensor_scalar(
                                    dst, z1[:], b1s[:, p : p + 1], 0.0, ADD, MAX
                                )
                        if p == 0:
                            h1_tiles[gB] = [None] * NPAIR
                        h1_tiles[gB][p] = h1

                # ---- stage C: L2 quad block for group gC ----
                if 0 <= gC < NG:
                    for hf in (0, 1):
                        chunk = 2 * gC + hf
                        s = chunk % 4
                        if s == 0:
                            z2 = z2p.tile([128, CH], F32, tag="z2",
                                          name=f"z2_{chunk // 4}")
                            z2_cur[0] = z2
                            # bias pre-fill (sets has_written for the bank)
                            nc.tensor.matmul(
                                z2[:], b2w[0:1, :], ones1[0:1, :],
                                start=True, stop=False,
                            )
                        z2 = z2_cur[0]
                        cs = slice(hf * CH, (hf + 1) * CH)
                        last_chunk = s == 3
                        # two data quads ((1-a)*W2^T·relu), two comp quads
                        # (a*W2^T·(W1·h0 + b1), bias part pre-filled)
                        for ws_t, tiles, is_comp in (
                            (w2s, h1_tiles[gC], False),
                            (w21s, h0_tiles[gC], True),
                        ):
                            for q0 in (0, 4):
                                for p in range(q0, q0 + 4):
                                    g = p % 4
                                    blk = (s * NPAIR + p) * 32
                                    nc.tensor.matmul(
                                        z2[32 * g : 32 * g + 32, :],
                                        ws_t[:, blk : blk + 32],
                                        tiles[p][:, cs],
                                        start=False,
                                        stop=(last_chunk and is_comp and p == 7),
                                        tile_position=(0, 32 * g),
                                    )
                    h0_tiles[gC] = None
                    h1_tiles[gC] = None

                # ---- stage C evac: one copy + one DMA per 4 chunks ----
                if 0 <= gC < NG and gC % 2 == 1:
                    bank = gC // 2
                    ob = obp.tile([128, CH], F32, tag="ob", name=f"ob_{bank}")
                    nc.vector.tensor_copy(ob[:], z2_cur[0][:])
                    nc.sync.dma_start(ot[:, bank * CH : (bank + 1) * CH], ob[:])

    nc.finalize()
    return nc


_prog = None


def _get_program():
    global _prog
    if _prog is None:
        _prog = _build_program()
    return _prog


def _shard_inputs(x, w0, w1, w2, b0, b1, b2):
    """Host-side relayout + t-sharding. Returns list of 8 in_maps."""
    x = np.asarray(x, np.float32)
    w0 = np.array(w0, np.float32)  # copy: we zero the adjacency diagonal
    w1 = np.asarray(w1, np.float32)
    w2 = np.asarray(w2, np.float32)
    b0 = np.asarray(b0, np.float32)
    b1 = np.asarray(b1, np.float32)
    b2 = np.asarray(b2, np.float32)

    # adjacency mask: variable t cannot see itself -> w0[t, :, t] = 0
    ar = np.arange(D)
    w0[ar, :, ar] = 0.0

    xt = np.ascontiguousarray(x.T).astype(NPDT)  # (128, 8192)

    in_maps = []
    for c in range(NCORES):
        ts_ = slice(c * TPC, (c + 1) * TPC)
        w0c, w1c, w2c = w0[ts_], w1[ts_], w2[ts_]
        b0cc, b1cc, b2cc = b0[ts_], b1[ts_], b2[ts_]

        # w0t: (128 j, pair*128 + [ta's 64 i | tb's 64 i])
        w0T = w0c.transpose(0, 2, 1)  # (16, 128 j, 64 i)
        w0t_ = np.ascontiguousarray(
            w0T.reshape(NPAIR, 2, D, H).transpose(2, 0, 1, 3).reshape(D, NPAIR * 128)
        ).astype(NPDT)

        # w1bd: per-pair 128x128 block-diagonal blocks
        bd1 = np.zeros((NPAIR, 128, 128), np.float32)
        for p in range(NPAIR):
            bd1[p, 0:H, 0:H] = w1c[2 * p].T
            bd1[p, H:128, H:128] = w1c[2 * p + 1].T
        w1bd_ = np.ascontiguousarray(
            bd1.transpose(1, 0, 2).reshape(128, NPAIR * 128)
        ).astype(NPDT)

        b0c_ = np.ascontiguousarray(b0cc.reshape(NPAIR, 128).T).astype(np.float32)
        b1c_ = np.ascontiguousarray(b1cc.reshape(NPAIR, 128).T).astype(np.float32)

        # per-pair [128, 4] L2 weight block (h1-row layout x output r=2v+o)
        w2blk = np.zeros((NPAIR, 128, 4), np.float32)
        for p in range(NPAIR):
            w2blk[p, 0:H, 0:2] = w2c[2 * p].T  # (64, 2)
            w2blk[p, H:128, 2:4] = w2c[2 * p + 1].T
        # compensation stationary a*(W1bd @ w2blk) and bias corr a*w2blk^T b1
        compblk = ALPHA * np.einsum("pjk,pkm->pjm", bd1, w2blk)  # (NPAIR,128,4)
        b1rows = b1cc.reshape(NPAIR, 128)
        corr = ALPHA * np.einsum("pkm,pk->pm", w2blk, b1rows)  # (NPAIR, 4)

        # w2z / w21z: [128, (s*NPAIR+p)*32 + 8s+4(p//4)+r]
        w2z_ = np.zeros((128, 4 * NPAIR * 32), np.float32)
        w21z_ = np.zeros((128, 4 * NPAIR * 32), np.float32)
        for s in range(4):
            for p in range(NPAIR):
                base = (s * NPAIR + p) * 32 + 8 * s + 4 * (p // 4)
                w2z_[:, base : base + 4] = (1.0 - ALPHA) * w2blk[p]
                w21z_[:, base : base + 4] = compblk[p]
        w2z_ = w2z_.astype(NPDT)
        w21z_ = w21z_.astype(NPDT)

        # b2st: col pi = 32*(p%4) + 8s + 4*(p//4) + r -> b2[2p+v, o] + corr
        b2st_ = np.zeros(128, np.float32)
        b2q = b2cc.reshape(NPAIR, 4)  # [p, r]
        for p in range(NPAIR):
            vals = b2q[p] + corr[p]
            for s in range(4):
                pi = 32 * (p % 4) + 8 * s + 4 * (p // 4)
                b2st_[pi : pi + 4] = vals
        b2st_ = b2st_[None, :].astype(NPDT)

        in_maps.append(
            {
                "xt": xt,
                "w0t": w0t_,
                "w1bd": w1bd_,
                "w2z": w2z_,
                "w21z": w21z_,
                "b0c": b0c_,
                "b1c": b1c_,
                "b2st": b2st_,
            }
        )
    return in_maps


def _unshard_outputs(results):
    out = np.empty((B, D, O), np.float32)
    for c in range(NCORES):
        ot = results[c]["ot"]  # (128, NBANK*512)
        # partition = 32g + 8s + 4q + 2v + o ; col = bank*512 + cc
        # batch = (4*bank + s)*512 + cc ; var = 8q + 2g + v
        arr = ot.reshape(4, 4, 2, 2, 2, NBANK, CH)  # [g,s,q,v,o,bank,cc]
        blk = arr.transpose(5, 1, 6, 2, 0, 3, 4).reshape(B, TPC, O)
        out[:, c * TPC : (c + 1) * TPC, :] = blk
    return out


def kernel(x, w0, w1, w2, b0, b1, b2):
    nc = _get_program()
    in_maps = _shard_inputs(x, w0, w1, w2, b0, b1, b2)
    res = bass_utils.run_bass_kernel_spmd(nc, in_maps, core_ids=list(range(NCORES)))
    return _unshard_outputs(res.results)


# revision 32
# speedup vs baseline: 1.4913x; 1.0637x over previous
"""Trainium2 Bass kernel for per-variable-MLP GNN message passing.

Model (reference):
    adj  = ones(D,D) - eye(D)                       # var t cannot see itself
    h0   = leaky_relu(einsum('tij,bj->bti', w0*adjmask, x) + b0)
    h1   = leaky_relu(einsum('tij,btj->bti', w1, h0) + b1)
    out  = einsum('tij,btj->bti', w2, h1) + b2      # (B, D, O)

Sharding: variable axis t (128) split across 8 cores (16 vars each); each
core sees the full batch. Vars processed in pairs (two 64-wide MLPs stacked
to fill the 128-wide PE array); activations live transposed (feature on
partition, batch on free).

v5 structure (vs the 146us baseline):
- Batch is processed in GROUPS of 1024 (2 psum banks); the L0 psum tile is
  [128,1024] so the ScalarE Prelu epilogue runs at FD=1024 (amortized
  per-op overhead). L1 psums stay [128,512].
- ALL pairs use the relu decomposition  leaky(v) = (1-a)relu(v) + a*v:
  h1 is stored as relu(z1+b1) -- a single fused op on EITHER VectorE
  (tensor_scalar add+max) or ScalarE (Relu activation), splitting the
  PSUM->SBUF crossing load across both engines; the a*W2^T(z1+b1) term
  is restored inside L2 via an extra matmul on h0 with the host-folded
  stationary a*(W1@W2z), plus a bias correction in the b2 pre-fill.
- L2 exploits PE column-tiling concurrency (4 col-groups issue within
  ~4ns of each other, measured 3x): pair p lands in col group p%4, and
  four consecutive batch chunks stack in rows 8s of each group, so one
  psum bank holds 4 chunks of finished output, fully using all 128
  partitions. Data + compensation matmuls run as 4-MM quads; one copy +
  one DMA per bank evacuates 4 chunks (1MB/core total output traffic).
- b2 (with the relu-conversion correction) is pre-filled into the L2
  bank by a K=1 matmul so the evacuation is a plain copy.
- Pipeline: stage A (L0, group g), B (L1, g-1) interleaved at pair
  granularity, then the stage C (L2, g-2) quad block, so the in-order PE
  queue never parks behind an epilogue.

Matmuls run in fp16 (1 col/cycle on the PE, fp32 accumulate in PSUM).
"""

import numpy as np

import concourse.bass as bass
import concourse.mybir as mybir
import concourse.tile as tile
from concourse import bacc, bass_utils

F32 = mybir.dt.float32
DT = mybir.dt.float16
NPDT = np.float16

B = 8192  # batch
D = 128  # num variables (t)
H = 64  # hidden
O = 2  # output dim per variable
NCORES = 8
TPC = D // NCORES  # vars per core = 16
NPAIR = TPC // 2  # 8
GCH = 1024  # batch group (2 psum banks for the L0 tile)
NG = B // GCH  # 8
CH = 512  # psum bank = 512 fp32
NBANK = B // (4 * CH)  # L2 output banks (4 chunks each) = 4
ALPHA = 0.01  # leaky_relu slope

Prelu = mybir.ActivationFunctionType.Prelu
Relu = mybir.ActivationFunctionType.Relu
ADD = mybir.AluOpType.add
MULT = mybir.AluOpType.mult
MAX = mybir.AluOpType.max

# h1 relu-crossing halves routed to ScalarE (the rest go to VectorE)
CV_SC = {(0, 0), (2, 1), (4, 1)}
# group-0 L0 epilogues routed to VectorE (pipeline fill; DVE idle then)
G0_DD = {5, 6}


def _build_program():
    nc = bacc.Bacc(trn_type="TRN2")

    xt = nc.dram_tensor("xt", (D, B), DT, kind="ExternalInput")
    w0t = nc.dram_tensor("w0t", (D, NPAIR * 128), DT, kind="ExternalInput")
    w1bd = nc.dram_tensor("w1bd", (128, NPAIR * 128), DT, kind="ExternalInput")
    # w2z / w21z: per (chunk-slot s, pair p) a [128,32] block, nonzero only
    # in cols 8s+4(p//4)+0..4, so pair p's output lands at psum partition
    # 32*(p%4) + 8s + 4*(p//4) + r when issued at tile_position (0,32*(p%4)).
    w2z = nc.dram_tensor("w2z", (128, 4 * NPAIR * 32), DT, kind="ExternalInput")
    w21z = nc.dram_tensor("w21z", (128, 4 * NPAIR * 32), DT, kind="ExternalInput")
    b0c = nc.dram_tensor("b0c", (128, NPAIR), F32, kind="ExternalInput")
    b1c = nc.dram_tensor("b1c", (128, NPAIR), F32, kind="ExternalInput")
    # b2 (+ relu-conversion correction) as a K=1 matmul stationary
    b2st = nc.dram_tensor("b2st", (1, 128), DT, kind="ExternalInput")
    # output: bank b (4 chunks) -> cols [512b, 512b+512); host unscrambles
    ot = nc.dram_tensor("ot", (128, NBANK * CH), F32, kind="ExternalOutput")

    with tile.TileContext(nc) as tc:
        with (
            tc.tile_pool(name="wp", bufs=1) as wp,
            tc.tile_pool(name="hp", bufs=18) as hp,
            tc.tile_pool(name="yp", bufs=4) as yp,
            tc.tile_pool(name="obp", bufs=2) as obp,
            tc.tile_pool(name="z0p", bufs=2, space="PSUM") as z0p,
            tc.tile_pool(name="z1p", bufs=3, space="PSUM") as z1p,
            tc.tile_pool(name="z2p", bufs=1, space="PSUM") as z2p,
        ):
            xs = wp.tile([D, B], DT)
            w0s = wp.tile([D, NPAIR * 128], DT)
            w1s = wp.tile([128, NPAIR * 128], DT)
            w2s = wp.tile([128, 4 * NPAIR * 32], DT)
            w21s = wp.tile([128, 4 * NPAIR * 32], DT)
            b0s = wp.tile([128, NPAIR], F32)
            b1s = wp.tile([128, NPAIR], F32)
            b2w = wp.tile([1, 128], DT)
            ones1 = wp.tile([1, CH], DT)
            nc.sync.dma_start(xs[:, 0:GCH], xt[:, 0:GCH])
            nc.sync.dma_start(w0s[:], w0t[:])
            nc.sync.dma_start(b0s[:], b0c[:])
            nc.sync.dma_start(w1s[:], w1bd[:])
            nc.sync.dma_start(b1s[:], b1c[:])
            nc.sync.dma_start(w2s[:], w2z[:])
            nc.sync.dma_start(w21s[:], w21z[:])
            nc.sync.dma_start(b2w[:], b2st[:])
            nc.vector.memset(ones1[:], 1.0)
            xs_loaded = 1  # groups staged so far

            # PE warmup: dummy matmuls with no input-DMA dependency so the
            # HAM clock-gate reaches 8/8 while the input DMAs run.
            warm = wp.tile([128, CH], DT, name="warm")
            nc.vector.memset(warm[:], 0.0)
            wps = z1p.tile([128, CH], F32, name="warmps", tag="z1")
            for _ in range(8):
                nc.tensor.matmul(wps[:], warm[:, 0:128], warm[:], start=True,
                                 stop=True)
            # preload the Prelu ACT table set during the input DMAs
            wact = wp.tile([128, 8], DT, name="wact")
            nc.scalar.activation(wact[:], wps[:, 0:8], Prelu, bias=0.0,
                                 scale=1.0, alpha=ALPHA)

            def leaky_big(dst, z, bias_col):
                """dst = leaky_relu(z + bias) on VectorE (2 ops)."""
                w = z.shape[-1]
                y = yp.tile([128, GCH], DT, tag="y", name="y")
                nc.vector.tensor_scalar_add(y[:, 0:w], z, bias_col)
                nc.vector.scalar_tensor_tensor(dst, y[:, 0:w], ALPHA, y[:, 0:w],
                                               MULT, MAX)

            h0_tiles = [None] * NG  # 8 tiles of [128,1024] per group
            h1_tiles = [None] * NG
            z2_cur = [None]  # current 4-chunk L2 psum bank

            for k in range(NG + 2):
                if xs_loaded < NG:
                    g = xs_loaded
                    nc.sync.dma_start(
                        xs[:, g * GCH : (g + 1) * GCH], xt[:, g * GCH : (g + 1) * GCH]
                    )
                    xs_loaded += 1
                gA, gB, gC = k, k - 1, k - 2

                # stage C emitter: one 4-MM quad per call, interleaved into
                # the A/B pair loop so the PE queue always has quad work to
                # fill epilogue-wait windows. Per group: 2 chunks x (2 data
                # + 2 comp) quads = 8 quads.
                def emit_quad(qi):
                    hf, half = divmod(qi, 4)
                    is_comp, q0 = divmod(half, 2)
                    chunk = 2 * gC + hf
                    s = chunk % 4
                    if qi == 0 and s == 0:
                        z2 = z2p.tile([128, CH], F32, tag="z2",
                                      name=f"z2_{chunk // 4}")
                        z2_cur[0] = z2
                        nc.tensor.matmul(
                            z2[:], b2w[0:1, :], ones1[0:1, :],
                            start=True, stop=False,
                        )
                    z2 = z2_cur[0]
                    cs = slice(hf * CH, (hf + 1) * CH)
                    ws_t = w21s if is_comp else w2s
                    tiles = h0_tiles[gC] if is_comp else h1_tiles[gC]
                    for p in range(4 * q0, 4 * q0 + 4):
                        g = p % 4
                        blk = (s * NPAIR + p) * 32
                        nc.tensor.matmul(
                            z2[32 * g : 32 * g + 32, :],
                            ws_t[:, blk : blk + 32],
                            tiles[p][:, cs],
                            start=False,
                            stop=(s == 3 and qi == 7 and p == 4 * q0 + 3),
                            tile_position=(0, 32 * g),
                        )

                for p in range(NPAIR):
                    # ---- stage C: one L2 quad per pair slot ----
                    if 0 <= gC < NG:
                        emit_quad(p)
                    # ---- stage A: L0 pair p of group gA ----
                    if gA < NG:
                        z0 = z0p.tile([128, GCH], F32, tag="z0",
                                      name=f"z0_{gA}_{p}")
                        for hf in (0, 1):
                            nc.tensor.matmul(
                                z0[:, hf * CH : (hf + 1) * CH],
                                w0s[:, bass.ts(p, 128)],
                                xs[:, gA * GCH + hf * CH : gA * GCH + (hf + 1) * CH],
                                start=True, stop=True,
                            )
                        h0 = hp.tile([128, GCH], DT, tag="h0",
                                     name=f"h0_{gA}_{p}", bufs=26)
                        if gA == 0 and p in G0_DD:
                            leaky_big(h0[:], z0[:], b0s[:, p : p + 1])
                        else:
                            nc.scalar.activation(
                                h0[:], z0[:], Prelu, bias=b0s[:, p : p + 1],
                                scale=1.0, alpha=ALPHA,
                            )
                        if p == 0:
                            h0_tiles[gA] = [None] * NPAIR
                        h0_tiles[gA][p] = h0

                    # ---- stage B: L1 pair p of group gB (relu crossing) ----
                    if 0 <= gB < NG:
                        h1 = hp.tile([128, GCH], DT, tag="h1",
                                     name=f"h1_{gB}_{p}", bufs=18)
                        for hf in (0, 1):
                            z1 = z1p.tile([128, CH], F32, tag="z1",
                                          name=f"z1_{gB}_{p}_{hf}")
                            nc.tensor.matmul(
                                z1[:], w1s[:, bass.ts(p, 128)],
                                h0_tiles[gB][p][:, hf * CH : (hf + 1) * CH],
                                start=True, stop=True,
                            )
                            dst = h1[:, hf * CH : (hf + 1) * CH]
                            if (p, hf) in CV_SC:
                                nc.scalar.activation(
                                    dst, z1[:], Relu, bias=b1s[:, p : p + 1],
                                    scale=1.0,
                                )
                            else:
                                nc.vector.tensor_scalar(
                                    dst, z1[:], b1s[:, p : p + 1], 0.0, ADD, MAX
                                )
                        if p == 0:
                            h1_tiles[gB] = [None] * NPAIR
                        h1_tiles[gB][p] = h1

                if 0 <= gC < NG:
                    h0_tiles[gC] = None
                    h1_tiles[gC] = None

                # ---- stage C evac: one copy + one DMA per 4 chunks ----
                if 0 <= gC < NG and gC % 2 == 1:
                    bank = gC // 2
                    ob = obp.tile([128, CH], F32, tag="ob", name=f"ob_{bank}")
                    if bank % 2 == 0:
                        nc.vector.tensor_copy(ob[:], z2_cur[0][:])
                    else:
                        nc.scalar.copy(ob[:], z2_cur[0][:])
                    nc.sync.dma_start(ot[:, bank * CH : (bank + 1) * CH], ob[:])

    nc.finalize()
    return nc


_prog = None


def _get_program():
    global _prog
    if _prog is None:
        _prog = _build_program()
    return _prog


def _shard_inputs(x, w0, w1, w2, b0, b1, b2):
    """Host-side relayout + t-sharding. Returns list of 8 in_maps."""
    x = np.asarray(x, np.float32)
    w0 = np.array(w0, np.float32)  # copy: we zero the adjacency diagonal
    w1 = np.asarray(w1, np.float32)
    w2 = np.asarray(w2, np.float32)
    b0 = np.asarray(b0, np.float32)
    b1 = np.asarray(b1, np.float32)
    b2 = np.asarray(b2, np.float32)

    # adjacency mask: variable t cannot see itself -> w0[t, :, t] = 0
    ar = np.arange(D)
    w0[ar, :, ar] = 0.0

    xt = np.ascontiguousarray(x.T).astype(NPDT)  # (128, 8192)

    in_maps = []
    for c in range(NCORES):
        ts_ = slice(c * TPC, (c + 1) * TPC)
        w0c, w1c, w2c = w0[ts_], w1[ts_], w2[ts_]
        b0cc, b1cc, b2cc = b0[ts_], b1[ts_], b2[ts_]

        # w0t: (128 j, pair*128 + [ta's 64 i | tb's 64 i])
        w0T = w0c.transpose(0, 2, 1)  # (16, 128 j, 64 i)
        w0t_ = np.ascontiguousarray(
            w0T.reshape(NPAIR, 2, D, H).transpose(2, 0, 1, 3).reshape(D, NPAIR * 128)
        ).astype(NPDT)

        # w1bd: per-pair 128x128 block-diagonal blocks
        bd1 = np.zeros((NPAIR, 128, 128), np.float32)
        for p in range(NPAIR):
            bd1[p, 0:H, 0:H] = w1c[2 * p].T
            bd1[p, H:128, H:128] = w1c[2 * p + 1].T
        w1bd_ = np.ascontiguousarray(
            bd1.transpose(1, 0, 2).reshape(128, NPAIR * 128)
        ).astype(NPDT)

        b0c_ = np.ascontiguousarray(b0cc.reshape(NPAIR, 128).T).astype(np.float32)
        b1c_ = np.ascontiguousarray(b1cc.reshape(NPAIR, 128).T).astype(np.float32)

        # per-pair [128, 4] L2 weight block (h1-row layout x output r=2v+o)
        w2blk = np.zeros((NPAIR, 128, 4), np.float32)
        for p in range(NPAIR):
            w2blk[p, 0:H, 0:2] = w2c[2 * p].T  # (64, 2)
            w2blk[p, H:128, 2:4] = w2c[2 * p + 1].T
        # compensation stationary a*(W1bd @ w2blk) and bias corr a*w2blk^T b1
        compblk = ALPHA * np.einsum("pjk,pkm->pjm", bd1, w2blk)  # (NPAIR,128,4)
        b1rows = b1cc.reshape(NPAIR, 128)
        corr = ALPHA * np.einsum("pkm,pk->pm", w2blk, b1rows)  # (NPAIR, 4)

        # w2z / w21z: [128, (s*NPAIR+p)*32 + 8s+4(p//4)+r]
        w2z_ = np.zeros((128, 4 * NPAIR * 32), np.float32)
        w21z_ = np.zeros((128, 4 * NPAIR * 32), np.float32)
        for s in range(4):
            for p in range(NPAIR):
                base = (s * NPAIR + p) * 32 + 8 * s + 4 * (p // 4)
                w2z_[:, base : base + 4] = (1.0 - ALPHA) * w2blk[p]
                w21z_[:, base : base + 4] = compblk[p]
        w2z_ = w2z_.astype(NPDT)
        w21z_ = w21z_.astype(NPDT)

        # b2st: col pi = 32*(p%4) + 8s + 4*(p//4) + r -> b2[2p+v, o] + corr
        b2st_ = np.zeros(128, np.float32)
        b2q = b2cc.reshape(NPAIR, 4)  # [p, r]
        for p in range(NPAIR):
            vals = b2q[p] + corr[p]
            for s in range(4):
                pi = 32 * (p % 4) + 8 * s + 4 * (p // 4)
                b2st_[pi : pi + 4] = vals
        b2st_ = b2st_[None, :].astype(NPDT)

        in_maps.append(
            {
                "xt": xt,
                "w0t": w0t_,
                "w1bd": w1bd_,
                "w2z": w2z_,
                "w21z": w21z_,
                "b0c": b0c_,
                "b1c": b1c_,
                "b2st": b2st_,
            }
        )
    return in_maps


def _unshard_outputs(results):
    out = np.empty((B, D, O), np.float32)
    for c in range(NCORES):
        ot = results[c]["ot"]  # (128, NBANK*512)
        # partition = 32g + 8s + 4q + 2v + o ; col = bank*512 + cc
        # batch = (4*bank + s)*512 + cc ; var = 8q + 2g + v
        arr = ot.reshape(4, 4, 2, 2, 2, NBANK, CH)  # [g,s,q,v,o,bank,cc]
        blk = arr.transpose(5, 1, 6, 2, 0, 3, 4).reshape(B, TPC, O)
        out[:, c * TPC : (c + 1) * TPC, :] = blk
    return out


def kernel(x, w0, w1, w2, b0, b1, b2):
    nc = _get_program()
    in_maps = _shard_inputs(x, w0, w1, w2, b0, b1, b2)
    res = bass_utils.run_bass_kernel_spmd(nc, in_maps, core_ids=list(range(NCORES)))
    return _unshard_outputs(res.results)


# revision 35
# speedup vs baseline: 1.5009x; 1.0064x over previous
"""Trainium2 Bass kernel for per-variable-MLP GNN message passing.

Model (reference):
    adj  = ones(D,D) - eye(D)                       # var t cannot see itself
    h0   = leaky_relu(einsum('tij,bj->bti', w0*adjmask, x) + b0)
    h1   = leaky_relu(einsum('tij,btj->bti', w1, h0) + b1)
    out  = einsum('tij,btj->bti', w2, h1) + b2      # (B, D, O)

Sharding: variable axis t (128) split across 8 cores (16 vars each); each
core sees the full batch. Vars processed in pairs (two 64-wide MLPs stacked
to fill the 128-wide PE array); activations live transposed (feature on
partition, batch on free).

v5 structure (vs the 146us baseline):
- Batch is processed in GROUPS of 1024 (2 psum banks); the L0 psum tile is
  [128,1024] so the ScalarE Prelu epilogue runs at FD=1024 (amortized
  per-op overhead). L1 psums stay [128,512].
- ALL pairs use the relu decomposition  leaky(v) = (1-a)relu(v) + a*v:
  h1 is stored as relu(z1+b1) -- a single fused op on EITHER VectorE
  (tensor_scalar add+max) or ScalarE (Relu activation), splitting the
  PSUM->SBUF crossing load across both engines; the a*W2^T(z1+b1) term
  is restored inside L2 via an extra matmul on h0 with the host-folded
  stationary a*(W1@W2z), plus a bias correction in the b2 pre-fill.
- L2 exploits PE column-tiling concurrency (4 col-groups issue within
  ~4ns of each other, measured 3x): pair p lands in col group p%4, and
  four consecutive batch chunks stack in rows 8s of each group, so one
  psum bank holds 4 chunks of finished output, fully using all 128
  partitions. Data + compensation matmuls run as 4-MM quads; one copy +
  one DMA per bank evacuates 4 chunks (1MB/core total output traffic).
- b2 (with the relu-conversion correction) is pre-filled into the L2
  bank by a K=1 matmul so the evacuation is a plain copy.
- Pipeline: stage A (L0, group g), B (L1, g-1) interleaved at pair
  granularity, then the stage C (L2, g-2) quad block, so the in-order PE
  queue never parks behind an epilogue.

Matmuls run in fp16 (1 col/cycle on the PE, fp32 accumulate in PSUM).
"""

import numpy as np

import concourse.bass as bass
import concourse.mybir as mybir
import concourse.tile as tile
from concourse import bacc, bass_utils

F32 = mybir.dt.float32
DT = mybir.dt.float16
NPDT = np.float16

B = 8192  # batch
D = 128  # num variables (t)
H = 64  # hidden
O = 2  # output dim per variable
NCORES = 8
TPC = D // NCORES  # vars per core = 16
NPAIR = TPC // 2  # 8
GCH = 1024  # batch group (2 psum banks for the L0 tile)
NG = B // GCH  # 8
CH = 512  # psum bank = 512 fp32
NBANK = B // (4 * CH)  # L2 output banks (4 chunks each) = 4
ALPHA = 0.01  # leaky_relu slope

Prelu = mybir.ActivationFunctionType.Prelu
Relu = mybir.ActivationFunctionType.Relu
ADD = mybir.AluOpType.add
MULT = mybir.AluOpType.mult
MAX = mybir.AluOpType.max

# h1 relu-crossing halves routed to ScalarE (the rest go to VectorE)
CV_SC = {(0, 0), (2, 1), (4, 1)}
# group-0 L0 epilogues routed to VectorE (pipeline fill; DVE idle then)
G0_DD = {5, 6}


def _build_program():
    nc = bacc.Bacc(trn_type="TRN2")

    xt = nc.dram_tensor("xt", (D, B), DT, kind="ExternalInput")
    w0t = nc.dram_tensor("w0t", (D, NPAIR * 128), DT, kind="ExternalInput")
    w1bd = nc.dram_tensor("w1bd", (128, NPAIR * 128), DT, kind="ExternalInput")
    # w2z / w21z: per (chunk-slot s, pair p) a [128,32] block, nonzero only
    # in cols 8s+4(p//4)+0..4, so pair p's output lands at psum partition
    # 32*(p%4) + 8s + 4*(p//4) + r when issued at tile_position (0,32*(p%4)).
    w2z = nc.dram_tensor("w2z", (128, 4 * NPAIR * 32), DT, kind="ExternalInput")
    w21z = nc.dram_tensor("w21z", (128, 4 * NPAIR * 32), DT, kind="ExternalInput")
    b0c = nc.dram_tensor("b0c", (128, NPAIR), F32, kind="ExternalInput")
    b1c = nc.dram_tensor("b1c", (128, NPAIR), F32, kind="ExternalInput")
    # b2 (+ relu-conversion correction) as a K=1 matmul stationary
    b2st = nc.dram_tensor("b2st", (1, 128), DT, kind="ExternalInput")
    # output: bank b (4 chunks) -> cols [512b, 512b+512); host unscrambles
    ot = nc.dram_tensor("ot", (128, NBANK * CH), F32, kind="ExternalOutput")

    with tile.TileContext(nc) as tc:
        with (
            tc.tile_pool(name="wp", bufs=1) as wp,
            tc.tile_pool(name="hp", bufs=18) as hp,
            tc.tile_pool(name="yp", bufs=4) as yp,
            tc.tile_pool(name="obp", bufs=2) as obp,
            tc.tile_pool(name="z0p", bufs=2, space="PSUM") as z0p,
            tc.tile_pool(name="z1p", bufs=3, space="PSUM") as z1p,
            tc.tile_pool(name="z2p", bufs=1, space="PSUM") as z2p,
        ):
            xs = wp.tile([D, B], DT)
            w0s = wp.tile([D, NPAIR * 128], DT)
            w1s = wp.tile([128, NPAIR * 128], DT)
            w2s = wp.tile([128, 4 * NPAIR * 32], DT)
            w21s = wp.tile([128, 4 * NPAIR * 32], DT)
            b0s = wp.tile([128, NPAIR], F32)
            b1s = wp.tile([128, NPAIR], F32)
            b2w = wp.tile([1, 128], DT)
            ones1 = wp.tile([1, CH], DT)
            nc.sync.dma_start(xs[:, 0:GCH], xt[:, 0:GCH])
            nc.sync.dma_start(w0s[:], w0t[:])
            nc.sync.dma_start(b0s[:], b0c[:])
            nc.sync.dma_start(w1s[:], w1bd[:])
            nc.sync.dma_start(b1s[:], b1c[:])
            nc.sync.dma_start(w2s[:], w2z[:])
            nc.sync.dma_start(w21s[:], w21z[:])
            nc.sync.dma_start(b2w[:], b2st[:])
            nc.vector.memset(ones1[:], 1.0)
            xs_loaded = 1  # groups staged so far

            # preload the Prelu ACT table set while the input DMAs run
            # (reads uninitialized SBUF -- only the table load matters)
            wact = wp.tile([128, 8], DT, name="wact")
            wsrc = wp.tile([128, 8], DT, name="wsrc")
            nc.vector.memset(wsrc[:], 0.0)
            nc.scalar.activation(wact[:], wsrc[:], Prelu, bias=0.0,
                                 scale=1.0, alpha=ALPHA)

            def leaky_big(dst, z, bias_col):
                """dst = leaky_relu(z + bias) on VectorE (2 ops)."""
                w = z.shape[-1]
                y = yp.tile([128, GCH], DT, tag="y", name="y")
                nc.vector.tensor_scalar_add(y[:, 0:w], z, bias_col)
                nc.vector.scalar_tensor_tensor(dst, y[:, 0:w], ALPHA, y[:, 0:w],
                                               MULT, MAX)

            h0_tiles = {}  # flat pair index -> [128,1024] tile
            h1_tiles = {}
            z2_cur = [None]  # current 4-chunk L2 psum bank
            NPT = NG * NPAIR  # 64 flat pair-steps
            LAG_B = 3  # stage B lags A by 3 pairs
            LAG_C = 8  # group g's 8 L2 quads run at steps g*8+8 .. g*8+15

            # L2 quad j of group g: ordered so quads needing pairs 4-7
            # (q0=1) come last. (hf, is_comp, q0)
            QUADS = [(0, 0, 0), (1, 0, 0), (0, 1, 0), (1, 1, 0),
                     (0, 0, 1), (1, 0, 1), (0, 1, 1), (1, 1, 1)]

            def emit_quad(g, j):
                hf, is_comp, q0 = QUADS[j]
                chunk = 2 * g + hf
                s = chunk % 4
                if j == 0 and s == 0:
                    z2 = z2p.tile([128, CH], F32, tag="z2",
                                  name=f"z2_{chunk // 4}")
                    z2_cur[0] = z2
                    # bias pre-fill (sets has_written for the whole bank)
                    nc.tensor.matmul(
                        z2[:], b2w[0:1, :], ones1[0:1, :],
                        start=True, stop=False,
                    )
                z2 = z2_cur[0]
                cs = slice(hf * CH, (hf + 1) * CH)
                ws_t = w21s if is_comp else w2s
                tiles = h0_tiles if is_comp else h1_tiles
                for p in range(4 * q0, 4 * q0 + 4):
                    cg = p % 4
                    blk = (s * NPAIR + p) * 32
                    nc.tensor.matmul(
                        z2[32 * cg : 32 * cg + 32, :],
                        ws_t[:, blk : blk + 32],
                        tiles[g * NPAIR + p][:, cs],
                        start=False,
                        stop=(s == 3 and j == 7 and p == 4 * q0 + 3),
                        tile_position=(0, 32 * cg),
                    )

            for i in range(NPT + LAG_C + NPAIR):
                # xs prefetch, one group ahead of stage A's needs
                if i % NPAIR == 0 and xs_loaded < NG and xs_loaded <= i // NPAIR + 1:
                    g = xs_loaded
                    nc.sync.dma_start(
                        xs[:, g * GCH : (g + 1) * GCH], xt[:, g * GCH : (g + 1) * GCH]
                    )
                    xs_loaded += 1

                # ---- stage C: one L2 quad per step ----
                iC = i - LAG_C
                if 0 <= iC < NPT:
                    emit_quad(iC // NPAIR, iC % NPAIR)

                # ---- stage A: L0 flat pair i ----
                if i < NPT:
                    gA, p = divmod(i, NPAIR)
                    z0 = z0p.tile([128, GCH], F32, tag="z0", name=f"z0_{i}")
                    for hf in (0, 1):
                        nc.tensor.matmul(
                            z0[:, hf * CH : (hf + 1) * CH],
                            w0s[:, bass.ts(p, 128)],
                            xs[:, gA * GCH + hf * CH : gA * GCH + (hf + 1) * CH],
                            start=True, stop=True,
                        )
                    h0 = hp.tile([128, GCH], DT, tag="h0", name=f"h0_{i}",
                                 bufs=20)
                    if i < NPAIR and p in G0_DD:
                        leaky_big(h0[:], z0[:], b0s[:, p : p + 1])
                    else:
                        nc.scalar.activation(
                            h0[:], z0[:], Prelu, bias=b0s[:, p : p + 1],
                            scale=1.0, alpha=ALPHA,
                        )
                    h0_tiles[i] = h0

                # ---- stage B: L1 flat pair i-LAG_B (relu crossing) ----
                iB = i - LAG_B
                if 0 <= iB < NPT:
                    gB, p = divmod(iB, NPAIR)
                    h1 = hp.tile([128, GCH], DT, tag="h1", name=f"h1_{iB}",
                                 bufs=14)
                    for hf in (0, 1):
                        z1 = z1p.tile([128, CH], F32, tag="z1",
                                      name=f"z1_{iB}_{hf}")
                        nc.tensor.matmul(
                            z1[:], w1s[:, bass.ts(p, 128)],
                            h0_tiles[iB][:, hf * CH : (hf + 1) * CH],
                            start=True, stop=True,
                        )
                        dst = h1[:, hf * CH : (hf + 1) * CH]
                        if (p, hf) in CV_SC and not (gB % 2 and (p, hf) == (2, 1)):
                            nc.scalar.activation(
                                dst, z1[:], Relu, bias=b1s[:, p : p + 1],
                                scale=1.0,
                            )
                        else:
                            nc.vector.tensor_scalar(
                                dst, z1[:], b1s[:, p : p + 1], 0.0, ADD, MAX
                            )
                    h1_tiles[iB] = h1

                # release tiles consumed by the just-emitted quad step
                if 0 <= iC < NPT and iC % NPAIR == NPAIR - 1:
                    g = iC // NPAIR
                    for p in range(NPAIR):
                        h0_tiles.pop(g * NPAIR + p, None)
                        h1_tiles.pop(g * NPAIR + p, None)

                # ---- stage C evac: one copy + one DMA per 4 chunks ----
                if 0 <= iC < NPT and iC % (2 * NPAIR) == 2 * NPAIR - 1:
                    bank = iC // (2 * NPAIR)
                    ob = obp.tile([128, CH], F32, tag="ob", name=f"ob_{bank}")
                    if bank % 2 == 0:
                        nc.vector.tensor_copy(ob[:], z2_cur[0][:])
                    else:
                        nc.scalar.copy(ob[:], z2_cur[0][:])
                    nc.sync.dma_start(ot[:, bank * CH : (bank + 1) * CH], ob[:])

    nc.finalize()
    return nc


_prog = None


def _get_program():
    global _prog
    if _prog is None:
        _prog = _build_program()
    return _prog


def _shard_inputs(x, w0, w1, w2, b0, b1, b2):
    """Host-side relayout + t-sharding. Returns list of 8 in_maps."""
    x = np.asarray(x, np.float32)
    w0 = np.array(w0, np.float32)  # copy: we zero the adjacency diagonal
    w1 = np.asarray(w1, np.float32)
    w2 = np.asarray(w2, np.float32)
    b0 = np.asarray(b0, np.float32)
    b1 = np.asarray(b1, np.float32)
    b2 = np.asarray(b2, np.float32)

    # adjacency mask: variable t cannot see itself -> w0[t, :, t] = 0
    ar = np.arange(D)
    w0[ar, :, ar] = 0.0

    xt = np.ascontiguousarray(x.T).astype(NPDT)  # (128, 8192)

    in_maps = []
    for c in range(NCORES):
        ts_ = slice(c * TPC, (c + 1) * TPC)
        w0c, w1c, w2c = w0[ts_], w1[ts_], w2[ts_]
        b0cc, b1cc, b2cc = b0[ts_], b1[ts_], b2[ts_]

        # w0t: (128 j, pair*128 + [ta's 64 i | tb's 64 i])
        w0T = w0c.transpose(0, 2, 1)  # (16, 128 j, 64 i)
        w0t_ = np.ascontiguousarray(
            w0T.reshape(NPAIR, 2, D, H).transpose(2, 0, 1, 3).reshape(D, NPAIR * 128)
        ).astype(NPDT)

        # w1bd: per-pair 128x128 block-diagonal blocks
        bd1 = np.zeros((NPAIR, 128, 128), np.float32)
        for p in range(NPAIR):
            bd1[p, 0:H, 0:H] = w1c[2 * p].T
            bd1[p, H:128, H:128] = w1c[2 * p + 1].T
        w1bd_ = np.ascontiguousarray(
            bd1.transpose(1, 0, 2).reshape(128, NPAIR * 128)
        ).astype(NPDT)

        b0c_ = np.ascontiguousarray(b0cc.reshape(NPAIR, 128).T).astype(np.float32)
        b1c_ = np.ascontiguousarray(b1cc.reshape(NPAIR, 128).T).astype(np.float32)

        # per-pair [128, 4] L2 weight block (h1-row layout x output r=2v+o)
        w2blk = np.zeros((NPAIR, 128, 4), np.float32)
        for p in range(NPAIR):
            w2blk[p, 0:H, 0:2] = w2c[2 * p].T  # (64, 2)
            w2blk[p, H:128, 2:4] = w2c[2 * p + 1].T
        # compensation stationary a*(W1bd @ w2blk) and bias corr a*w2blk^T b1
        compblk = ALPHA * np.einsum("pjk,pkm->pjm", bd1, w2blk)  # (NPAIR,128,4)
        b1rows = b1cc.reshape(NPAIR, 128)
        corr = ALPHA * np.einsum("pkm,pk->pm", w2blk, b1rows)  # (NPAIR, 4)

        # w2z / w21z: [128, (s*NPAIR+p)*32 + 8s+4(p//4)+r]
        w2z_ = np.zeros((128, 4 * NPAIR * 32), np.float32)
        w21z_ = np.zeros((128, 4 * NPAIR * 32), np.float32)
        for s in range(4):
            for p in range(NPAIR):
                base = (s * NPAIR + p) * 32 + 8 * s + 4 * (p // 4)
                w2z_[:, base : base + 4] = (1.0 - ALPHA) * w2blk[p]
                w21z_[:, base : base + 4] = compblk[p]
        w2z_ = w2z_.astype(NPDT)
        w21z_ = w21z_.astype(NPDT)

        # b2st: col pi = 32*(p%4) + 8s + 4*(p//4) + r -> b2[2p+v, o] + corr
        b2st_ = np.zeros(128, np.float32)
        b2q = b2cc.reshape(NPAIR, 4)  # [p, r]
        for p in range(NPAIR):
            vals = b2q[p] + corr[p]
            for s in range(4):
                pi = 32 * (p % 4) + 8 * s + 4 * (p // 4)
                b2st_[pi : pi + 4] = vals
        b2st_ = b2st_[None, :].astype(NPDT)

        in_maps.append(
            {
                "xt": xt,
                "w0t": w0t_,
                "w1bd": w1bd_,
                "w2z": w2z_,
                "w21z": w21z_,
                "b0c": b0c_,
                "b1c": b1c_,
                "b2st": b2st_,
            }
        )
    return in_maps


def _unshard_outputs(results):
    out = np.empty((B, D, O), np.float32)
    for c in range(NCORES):
        ot = results[c]["ot"]  # (128, NBANK*512)
        # partition = 32g + 8s + 4q + 2v + o ; col = bank*512 + cc
        # batch = (4*bank + s)*512 + cc ; var = 8q + 2g + v
        arr = ot.reshape(4, 4, 2, 2, 2, NBANK, CH)  # [g,s,q,v,o,bank,cc]
        blk = arr.transpose(5, 1, 6, 2, 0, 3, 4).reshape(B, TPC, O)
        out[:, c * TPC : (c + 1) * TPC, :] = blk
    return out


def kernel(x, w0, w1, w2, b0, b1, b2):
    nc = _get_program()
    in_maps = _shard_inputs(x, w0, w1, w2, b0, b1, b2)
    res = bass_utils.run_bass_kernel_spmd(nc, in_maps, core_ids=list(range(NCORES)))
    return _unshard_outputs(res.results)


# revision 36
# speedup vs baseline: 1.5195x; 1.0124x over previous
"""Trainium2 Bass kernel for per-variable-MLP GNN message passing.

Model (reference):
    adj  = ones(D,D) - eye(D)                       # var t cannot see itself
    h0   = leaky_relu(einsum('tij,bj->bti', w0*adjmask, x) + b0)
    h1   = leaky_relu(einsum('tij,btj->bti', w1, h0) + b1)
    out  = einsum('tij,btj->bti', w2, h1) + b2      # (B, D, O)

Sharding: variable axis t (128) split across 8 cores (16 vars each); each
core sees the full batch. Vars processed in pairs (two 64-wide MLPs stacked
to fill the 128-wide PE array); activations live transposed (feature on
partition, batch on free).

v5 structure (vs the 146us baseline):
- Batch is processed in GROUPS of 1024 (2 psum banks); the L0 psum tile is
  [128,1024] so the ScalarE Prelu epilogue runs at FD=1024 (amortized
  per-op overhead). L1 psums stay [128,512].
- ALL pairs use the relu decomposition  leaky(v) = (1-a)relu(v) + a*v:
  h1 is stored as relu(z1+b1) -- a single fused op on EITHER VectorE
  (tensor_scalar add+max) or ScalarE (Relu activation), splitting the
  PSUM->SBUF crossing load across both engines; the a*W2^T(z1+b1) term
  is restored inside L2 via an extra matmul on h0 with the host-folded
  stationary a*(W1@W2z), plus a bias correction in the b2 pre-fill.
- L2 exploits PE column-tiling concurrency (4 col-groups issue within
  ~4ns of each other, measured 3x): pair p lands in col group p%4, and
  four consecutive batch chunks stack in rows 8s of each group, so one
  psum bank holds 4 chunks of finished output, fully using all 128
  partitions. Data + compensation matmuls run as 4-MM quads; one copy +
  one DMA per bank evacuates 4 chunks (1MB/core total output traffic).
- b2 (with the relu-conversion correction) is pre-filled into the L2
  bank by a K=1 matmul so the evacuation is a plain copy.
- Pipeline: stage A (L0, group g), B (L1, g-1) interleaved at pair
  granularity, then the stage C (L2, g-2) quad block, so the in-order PE
  queue never parks behind an epilogue.

Matmuls run in fp16 (1 col/cycle on the PE, fp32 accumulate in PSUM).
"""

import numpy as np

import concourse.bass as bass
import concourse.mybir as mybir
import concourse.tile as tile
from concourse import bacc, bass_utils

F32 = mybir.dt.float32
DT = mybir.dt.float16
NPDT = np.float16

B = 8192  # batch
D = 128  # num variables (t)
H = 64  # hidden
O = 2  # output dim per variable
NCORES = 8
TPC = D // NCORES  # vars per core = 16
NPAIR = TPC // 2  # 8
GCH = 1024  # batch group (2 psum banks for the L0 tile)
NG = B // GCH  # 8
CH = 512  # psum bank = 512 fp32
NBANK = B // (4 * CH)  # L2 output banks (4 chunks each) = 4
ALPHA = 0.01  # leaky_relu slope

Prelu = mybir.ActivationFunctionType.Prelu
Relu = mybir.ActivationFunctionType.Relu
ADD = mybir.AluOpType.add
MULT = mybir.AluOpType.mult
MAX = mybir.AluOpType.max

# h1 relu-crossing halves routed to ScalarE (the rest go to VectorE)
CV_SC = {(0, 0), (2, 1), (4, 1)}
# group-0 L0 epilogues routed to VectorE (pipeline fill; DVE idle then)
G0_DD = {5, 6}


def _build_program():
    nc = bacc.Bacc(trn_type="TRN2")

    xt = nc.dram_tensor("xt", (D, B), DT, kind="ExternalInput")
    w0t = nc.dram_tensor("w0t", (D, NPAIR * 128), DT, kind="ExternalInput")
    w1bd = nc.dram_tensor("w1bd", (128, NPAIR * 128), DT, kind="ExternalInput")
    # w2z / w21z: per (chunk-slot s, pair p) a [128,32] block, nonzero only
    # in cols 8s+4(p//4)+0..4, so pair p's output lands at psum partition
    # 32*(p%4) + 8s + 4*(p//4) + r when issued at tile_position (0,32*(p%4)).
    w2z = nc.dram_tensor("w2z", (128, 4 * NPAIR * 32), DT, kind="ExternalInput")
    w21z = nc.dram_tensor("w21z", (128, 4 * NPAIR * 32), DT, kind="ExternalInput")
    b0c = nc.dram_tensor("b0c", (128, NPAIR), F32, kind="ExternalInput")
    b1c = nc.dram_tensor("b1c", (128, NPAIR), F32, kind="ExternalInput")
    # b2 (+ relu-conversion correction) as a K=1 matmul stationary
    b2st = nc.dram_tensor("b2st", (1, 128), DT, kind="ExternalInput")
    # output: bank b (4 chunks) -> cols [512b, 512b+512); host unscrambles
    ot = nc.dram_tensor("ot", (128, NBANK * CH), F32, kind="ExternalOutput")

    with tile.TileContext(nc) as tc:
        with (
            tc.tile_pool(name="wp", bufs=1) as wp,
            tc.tile_pool(name="hp", bufs=18) as hp,
            tc.tile_pool(name="yp", bufs=4) as yp,
            tc.tile_pool(name="obp", bufs=2) as obp,
            tc.tile_pool(name="z0p", bufs=2, space="PSUM") as z0p,
            tc.tile_pool(name="z1p", bufs=3, space="PSUM") as z1p,
            tc.tile_pool(name="z2p", bufs=1, space="PSUM") as z2p,
        ):
            xs = wp.tile([D, B], DT)
            w0s = wp.tile([D, NPAIR * 128], DT)
            w1s = wp.tile([128, NPAIR * 128], DT)
            w2s = wp.tile([128, 4 * NPAIR * 32], DT)
            w21s = wp.tile([128, 4 * NPAIR * 32], DT)
            b0s = wp.tile([128, NPAIR], F32)
            b1s = wp.tile([128, NPAIR], F32)
            b2w = wp.tile([1, 128], DT)
            ones1 = wp.tile([1, CH], DT)
            nc.sync.dma_start(xs[:, 0:GCH], xt[:, 0:GCH])
            nc.sync.dma_start(w0s[:], w0t[:])
            nc.sync.dma_start(b0s[:], b0c[:])
            nc.sync.dma_start(w1s[:], w1bd[:])
            nc.sync.dma_start(b1s[:], b1c[:])
            nc.sync.dma_start(w2s[:], w2z[:])
            nc.sync.dma_start(w21s[:], w21z[:])
            nc.sync.dma_start(b2w[:], b2st[:])
            nc.vector.memset(ones1[:], 1.0)
            xs_loaded = 1  # groups staged so far

            # preload the Prelu ACT table set while the input DMAs run
            # (reads uninitialized SBUF -- only the table load matters)
            wact = wp.tile([128, 8], DT, name="wact")
            wsrc = wp.tile([128, 8], DT, name="wsrc")
            nc.vector.memset(wsrc[:], 0.0)
            nc.scalar.activation(wact[:], wsrc[:], Prelu, bias=0.0,
                                 scale=1.0, alpha=ALPHA)
            # PE warmup: dummy matmuls with no input-DMA dependency keep the
            # PE busy through the DMA staging window and reach HAM 8/8.
            warm = wp.tile([128, CH], DT, name="warm")
            nc.vector.memset(warm[:], 0.0)
            wps = z1p.tile([128, CH], F32, name="warmps", tag="z1")
            for _ in range(6):
                nc.tensor.matmul(wps[:], warm[:, 0:128], warm[:], start=True,
                                 stop=True)

            def leaky_big(dst, z, bias_col):
                """dst = leaky_relu(z + bias) on VectorE (2 ops)."""
                w = z.shape[-1]
                y = yp.tile([128, GCH], DT, tag="y", name="y")
                nc.vector.tensor_scalar_add(y[:, 0:w], z, bias_col)
                nc.vector.scalar_tensor_tensor(dst, y[:, 0:w], ALPHA, y[:, 0:w],
                                               MULT, MAX)

            h0_tiles = {}  # flat pair index -> [128,1024] tile
            h1_tiles = {}
            z2_cur = [None]  # current 4-chunk L2 psum bank
            NPT = NG * NPAIR  # 64 flat pair-steps
            LAG_B = 3  # stage B lags A by 3 pairs
            LAG_C = 8  # group g's 8 L2 quads run at steps g*8+8 .. g*8+15

            # L2 quad j of group g: ordered so quads needing pairs 4-7
            # (q0=1) come last. (hf, is_comp, q0)
            QUADS = [(0, 0, 0), (1, 0, 0), (0, 1, 0), (1, 1, 0),
                     (0, 0, 1), (1, 0, 1), (0, 1, 1), (1, 1, 1)]

            def emit_quad(g, j):
                hf, is_comp, q0 = QUADS[j]
                chunk = 2 * g + hf
                s = chunk % 4
                if j == 0 and s == 0:
                    z2 = z2p.tile([128, CH], F32, tag="z2",
                                  name=f"z2_{chunk // 4}")
                    z2_cur[0] = z2
                    # bias pre-fill (sets has_written for the whole bank)
                    nc.tensor.matmul(
                        z2[:], b2w[0:1, :], ones1[0:1, :],
                        start=True, stop=False,
                    )
                z2 = z2_cur[0]
                cs = slice(hf * CH, (hf + 1) * CH)
                ws_t = w21s if is_comp else w2s
                tiles = h0_tiles if is_comp else h1_tiles
                for p in range(4 * q0, 4 * q0 + 4):
                    cg = p % 4
                    blk = (s * NPAIR + p) * 32
                    nc.tensor.matmul(
                        z2[32 * cg : 32 * cg + 32, :],
                        ws_t[:, blk : blk + 32],
                        tiles[g * NPAIR + p][:, cs],
                        start=False,
                        stop=(s == 3 and j == 7 and p == 4 * q0 + 3),
                        tile_position=(0, 32 * cg),
                    )

            for i in range(NPT + LAG_C + NPAIR):
                # xs prefetch, one group ahead of stage A's needs
                if i % NPAIR == 0 and xs_loaded < NG and xs_loaded <= i // NPAIR + 1:
                    g = xs_loaded
                    nc.sync.dma_start(
                        xs[:, g * GCH : (g + 1) * GCH], xt[:, g * GCH : (g + 1) * GCH]
                    )
                    xs_loaded += 1

                # ---- stage C: one L2 quad per step ----
                iC = i - LAG_C
                if 0 <= iC < NPT:
                    emit_quad(iC // NPAIR, iC % NPAIR)

                # ---- stage A: L0 flat pair i ----
                if i < NPT:
                    gA, p = divmod(i, NPAIR)
                    z0 = z0p.tile([128, GCH], F32, tag="z0", name=f"z0_{i}")
                    for hf in (0, 1):
                        nc.tensor.matmul(
                            z0[:, hf * CH : (hf + 1) * CH],
                            w0s[:, bass.ts(p, 128)],
                            xs[:, gA * GCH + hf * CH : gA * GCH + (hf + 1) * CH],
                            start=True, stop=True,
                        )
                    h0 = hp.tile([128, GCH], DT, tag="h0", name=f"h0_{i}",
                                 bufs=20)
                    if i < NPAIR and p in G0_DD:
                        leaky_big(h0[:], z0[:], b0s[:, p : p + 1])
                    else:
                        nc.scalar.activation(
                            h0[:], z0[:], Prelu, bias=b0s[:, p : p + 1],
                            scale=1.0, alpha=ALPHA,
                        )
                    h0_tiles[i] = h0

                # ---- stage B: L1 flat pair i-LAG_B (relu crossing) ----
                iB = i - LAG_B
                if 0 <= iB < NPT:
                    gB, p = divmod(iB, NPAIR)
                    h1 = hp.tile([128, GCH], DT, tag="h1", name=f"h1_{iB}",
                                 bufs=14)
                    for hf in (0, 1):
                        z1 = z1p.tile([128, CH], F32, tag="z1",
                                      name=f"z1_{iB}_{hf}")
                        nc.tensor.matmul(
                            z1[:], w1s[:, bass.ts(p, 128)],
                            h0_tiles[iB][:, hf * CH : (hf + 1) * CH],
                            start=True, stop=True,
                        )
                        dst = h1[:, hf * CH : (hf + 1) * CH]
                        if (p, hf) in CV_SC and not (gB % 2 and (p, hf) == (2, 1)):
                            nc.scalar.activation(
                                dst, z1[:], Relu, bias=b1s[:, p : p + 1],
                                scale=1.0,
                            )
                        else:
                            nc.vector.tensor_scalar(
                                dst, z1[:], b1s[:, p : p + 1], 0.0, ADD, MAX
                            )
                    h1_tiles[iB] = h1

                # release tiles consumed by the just-emitted quad step
                if 0 <= iC < NPT and iC % NPAIR == NPAIR - 1:
                    g = iC // NPAIR
                    for p in range(NPAIR):
                        h0_tiles.pop(g * NPAIR + p, None)
                        h1_tiles.pop(g * NPAIR + p, None)

                # ---- stage C evac: one copy + one DMA per 4 chunks ----
                if 0 <= iC < NPT and iC % (2 * NPAIR) == 2 * NPAIR - 1:
                    bank = iC // (2 * NPAIR)
                    ob = obp.tile([128, CH], F32, tag="ob", name=f"ob_{bank}")
                    if bank % 2 == 0:
                        nc.vector.tensor_copy(ob[:], z2_cur[0][:])
                    else:
                        nc.scalar.copy(ob[:], z2_cur[0][:])
                    nc.sync.dma_start(ot[:, bank * CH : (bank + 1) * CH], ob[:])

    nc.finalize()
    return nc


_prog = None


def _get_program():
    global _prog
    if _prog is None:
        _prog = _build_program()
    return _prog


def _shard_inputs(x, w0, w1, w2, b0, b1, b2):
    """Host-side relayout + t-sharding. Returns list of 8 in_maps."""
    x = np.asarray(x, np.float32)
    w0 = np.array(w0, np.float32)  # copy: we zero the adjacency diagonal
    w1 = np.asarray(w1, np.float32)
    w2 = np.asarray(w2, np.float32)
    b0 = np.asarray(b0, np.float32)
    b1 = np.asarray(b1, np.float32)
    b2 = np.asarray(b2, np.float32)

    # adjacency mask: variable t cannot see itself -> w0[t, :, t] = 0
    ar = np.arange(D)
    w0[ar, :, ar] = 0.0

    xt = np.ascontiguousarray(x.T).astype(NPDT)  # (128, 8192)

    in_maps = []
    for c in range(NCORES):
        ts_ = slice(c * TPC, (c + 1) * TPC)
        w0c, w1c, w2c = w0[ts_], w1[ts_], w2[ts_]
        b0cc, b1cc, b2cc = b0[ts_], b1[ts_], b2[ts_]

        # w0t: (128 j, pair*128 + [ta's 64 i | tb's 64 i])
        w0T = w0c.transpose(0, 2, 1)  # (16, 128 j, 64 i)
        w0t_ = np.ascontiguousarray(
            w0T.reshape(NPAIR, 2, D, H).transpose(2, 0, 1, 3).reshape(D, NPAIR * 128)
        ).astype(NPDT)

        # w1bd: per-pair 128x128 block-diagonal blocks
        bd1 = np.zeros((NPAIR, 128, 128), np.float32)
        for p in range(NPAIR):
            bd1[p, 0:H, 0:H] = w1c[2 * p].T
            bd1[p, H:128, H:128] = w1c[2 * p + 1].T
        w1bd_ = np.ascontiguousarray(
            bd1.transpose(1, 0, 2).reshape(128, NPAIR * 128)
        ).astype(NPDT)

        b0c_ = np.ascontiguousarray(b0cc.reshape(NPAIR, 128).T).astype(np.float32)
        b1c_ = np.ascontiguousarray(b1cc.reshape(NPAIR, 128).T).astype(np.float32)

        # per-pair [128, 4] L2 weight block (h1-row layout x output r=2v+o)
        w2blk = np.zeros((NPAIR, 128, 4), np.float32)
        for p in range(NPAIR):
            w2blk[p, 0:H, 0:2] = w2c[2 * p].T  # (64, 2)
            w2blk[p, H:128, 2:4] = w2c[2 * p + 1].T
        # compensation stationary a*(W1bd @ w2blk) and bias corr a*w2blk^T b1
        compblk = ALPHA * np.einsum("pjk,pkm->pjm", bd1, w2blk)  # (NPAIR,128,4)
        b1rows = b1cc.reshape(NPAIR, 128)
        corr = ALPHA * np.einsum("pkm,pk->pm", w2blk, b1rows)  # (NPAIR, 4)

        # w2z / w21z: [128, (s*NPAIR+p)*32 + 8s+4(p//4)+r]
        w2z_ = np.zeros((128, 4 * NPAIR * 32), np.float32)
        w21z_ = np.zeros((128, 4 * NPAIR * 32), np.float32)
        for s in range(4):
            for p in range(NPAIR):
                base = (s * NPAIR + p) * 32 + 8 * s + 4 * (p // 4)
                w2z_[:, base : base + 4] = (1.0 - ALPHA) * w2blk[p]
                w21z_[:, base : base + 4] = compblk[p]
        w2z_ = w2z_.astype(NPDT)
        w21z_ = w21z_.astype(NPDT)

        # b2st: col pi = 32*(p%4) + 8s + 4*(p//4) + r -> b2[2p+v, o] + corr
        b2st_ = np.zeros(128, np.float32)
        b2q = b2cc.reshape(NPAIR, 4)  # [p, r]
        for p in range(NPAIR):
            vals = b2q[p] + corr[p]
            for s in range(4):
                pi = 32 * (p % 4) + 8 * s + 4 * (p // 4)
                b2st_[pi : pi + 4] = vals
        b2st_ = b2st_[None, :].astype(NPDT)

        in_maps.append(
            {
                "xt": xt,
                "w0t": w0t_,
                "w1bd": w1bd_,
                "w2z": w2z_,
                "w21z": w21z_,
                "b0c": b0c_,
                "b1c": b1c_,
                "b2st": b2st_,
            }
        )
    return in_maps


def _unshard_outputs(results):
    out = np.empty((B, D, O), np.float32)
    for c in range(NCORES):
        ot = results[c]["ot"]  # (128, NBANK*512)
        # partition = 32g + 8s + 4q + 2v + o ; col = bank*512 + cc
        # batch = (4*bank + s)*512 + cc ; var = 8q + 2g + v
        arr = ot.reshape(4, 4, 2, 2, 2, NBANK, CH)  # [g,s,q,v,o,bank,cc]
        blk = arr.transpose(5, 1, 6, 2, 0, 3, 4).reshape(B, TPC, O)
        out[:, c * TPC : (c + 1) * TPC, :] = blk
    return out


def kernel(x, w0, w1, w2, b0, b1, b2):
    nc = _get_program()
    in_maps = _shard_inputs(x, w0, w1, w2, b0, b1, b2)
    res = bass_utils.run_bass_kernel_spmd(nc, in_maps, core_ids=list(range(NCORES)))
    return _unshard_outputs(res.results)
